# revision 55
# baseline (speedup 1.0000x reference)
"""Trainium2 Bass kernel for a single-layer MHA decode step with KV cache.

Problem (hardcoded from spec):
  x            [32, 8, 2048]      query tokens (B=32 batches x T=8 steps)
  cache_keys   [32, 32, 1016, 64] (B, H, S_cache, Dh)
  cache_values [32, 32, 1016, 64]
  Wq/Wk/Wv/Wo  [2048, 2048], biases [2048]
  out = MHA(x, cache) @ Wo.T + bo   -> [32, 8, 2048]

Sharding: tensor-parallel over heads. Each of the 8 cores handles 4 heads:
QKV projections for its head slice, attention over its KV-cache slice, and a
partial output projection (rank-256 slice of Wo). Host sums the 8 partials.

Design notes (v2 - transposed attention, bf16 streaming):
 - Everything DMA'd from DRAM is bf16: KV cache, weights, x, and the output
   partials. The kernel is HBM-bound on the KV cache (~33 MB/core in bf16),
   so halving wire bytes halves runtime; bf16 keeps rel-err ~5e-3 under the
   2e-2 gate. Cost-model time 120.9 us with the DMA device 92% busy at its
   modeled 360 GB/s (111.2 us of transfers = the bf16 byte floor).
 - Scores are computed TRANSPOSED: scT[s, (h,t)] per batch, with the key
   tile as the stationary matmul operand and the block-diagonal q as the
   moving operand (free dim 32).  s lives on partitions as s = 8p + i with
   i = 0..7 the free-dim chunk; p = 127 holds the 8 freshly projected keys
   (token 8b+i at chunk i), copied into the key tile on device.
 - Softmax: no max subtraction (scores are O(1), exp is safe in f32/bf16).
   exp runs on ACT into bf16 attnT; the normalizer Z per query is a
   ones-vector matmul over partitions; normalization is deferred to the
   psav evacuation (out = (1/Z) * sum exp*v factorizes).
 - AV is also transposed: out[dh, t] with v stationary, attn moving
   (free dim 16 covering a pair of heads), accumulated over the 8 s-chunks
   into one persistent [128, 1024] PSUM region laid out as aoT.
 - Output projection reads aoT directly; partials are written as bf16 and
   summed on host. It runs per token-half: half 0 mid-stream (hidden),
   half 1 in the tail.
 - Issue queues: kt on SP (HWDGE), vt + vnew on Pool (SWDGE, no HWDGE
   contention), exp/evac on ACT. Each DMA issue holds its queue's SEQ
   through desc-gen (~1.2 us), so one queue cannot feed both big streams.
 - Tail: the last 4 batches' key tiles load up front and their score
   chains run early, so after the final vt transfer only AV + a 32-column
   evac + the half-1 projection remain. 14 dummy matmuls at t=0 warm the
   PE p-state so QKV runs at full clock.
"""

import numpy as np
from ml_dtypes import bfloat16

import concourse.bass as bass
import concourse.mybir as mybir
import concourse.tile as tile
from concourse import bacc
from concourse import bass_utils

F32 = mybir.dt.float32
F16 = mybir.dt.float16
BF16 = mybir.dt.bfloat16

B, T, D = 32, 8, 2048
H, DH = 32, 64
S_CACHE, S = 1016, 1024
N_CORES = 8
HC = H // N_CORES          # heads per core = 4
TOK = B * T                # 256
QD = HC * DH               # 256 per-core qkv dims
P = 127                    # s-rows per chunk from the cache (1016 = 8*127)

AF = mybir.ActivationFunctionType
ALU = mybir.AluOpType
AX = mybir.AxisListType

CFG = {"dtype": "bf16", "bo_zero": True}


def build_nc(cfg=CFG):
    bo_zero = cfg.get("bo_zero", False)
    nc = bacc.Bacc(None, target_bir_lowering=False)

    xT = nc.dram_tensor("xT", [128, 16, 256], BF16, kind="ExternalInput")
    wqT = nc.dram_tensor("wqT", [128, 16, 256], BF16, kind="ExternalInput")
    wkT = nc.dram_tensor("wkT", [128, 16, 256], BF16, kind="ExternalInput")
    wvT = nc.dram_tensor("wvT", [128, 16, 256], BF16, kind="ExternalInput")
    woT = nc.dram_tensor("woT", [128, 2, 2048], BF16, kind="ExternalInput")
    bq = nc.dram_tensor("bq", [256], F32, kind="ExternalInput")
    bk = nc.dram_tensor("bk", [256], F32, kind="ExternalInput")
    bv = nc.dram_tensor("bv", [256], F32, kind="ExternalInput")
    bo = nc.dram_tensor("bo", [2048], BF16, kind="ExternalInput")
    # kT[b, (h,j), m, i, p]: keys with s = 8p+i on tile axes; p=127 is filled
    # on device with the new key of token 8b+i.
    kT = nc.dram_tensor("kT", [B, 128, 2, 8, 128], BF16, kind="ExternalInput")
    # v[b, p, i, h, dh] = cache_values[b, h, 8p+i, dh]
    v = nc.dram_tensor("v", [B, P, 8, HC, DH], BF16, kind="ExternalInput")
    out = nc.dram_tensor("out", [TOK, D], BF16, kind="ExternalOutput")

    with tile.TileContext(nc) as tc:
        with (
            tc.tile_pool(name="singles", bufs=1) as singles,
            tc.tile_pool(name="stream", bufs=8) as stream,
            tc.tile_pool(name="small", bufs=8) as small,
            tc.tile_pool(name="ps", bufs=6, space="PSUM") as ps,
            tc.tile_pool(name="ps_av", bufs=1, space="PSUM") as ps_av,
        ):
            # ---- persistent tiles ----
            xT_sb = singles.tile([128, 16, 256], BF16)
            wq_sb = singles.tile([128, 16, 256], BF16)
            wk_sb = singles.tile([128, 16, 256], BF16)
            wv_sb = singles.tile([128, 16, 256], BF16)
            wo_sb = singles.tile([128, 2, 2048], BF16)
            # weight loads split across issue queues: SP and Act alternate so
            # the shared HWDGE stage doesn't serialize one queue's prefetch
            nc.sync.dma_start(xT_sb, xT[:, :, :])
            nc.scalar.dma_start(wq_sb, wqT[:, :, :])
            nc.sync.dma_start(wk_sb, wkT[:, :, :])
            nc.scalar.dma_start(wv_sb, wvT[:, :, :])
            bq_sb = singles.tile([128, 2], F32)
            bk_sb = singles.tile([128, 2], F32)
            nc.gpsimd.dma_start(bq_sb, bq[:].rearrange("(m p) -> p m", p=128))
            nc.gpsimd.dma_start(bk_sb, bk[:].rearrange("(m p) -> p m", p=128))
            bv_bc = singles.tile([128, 256], F32)
            nc.gpsimd.dma_start(
                bv_bc, bass.AP(tensor=bv[:].tensor, offset=0, ap=[[0, 128], [1, 256]])
            )
            nc.gpsimd.dma_start(wo_sb, woT[:, :, :])
            # the LAST batches' keys load up front: their scores/exp/Z/recip
            # run early, so the tail after the final vt transfers is just
            # AV + evac + projection (no softmax chain on the critical path)
            HOIST = [28, 29, 30, 31]
            kt_h = {}
            for b in HOIST:
                kt_h[b] = singles.tile([128, 2, 8, 128], BF16, name=f"kt_h{b}")
                nc.sync.dma_start(kt_h[b], kT[b])
            if not bo_zero:
                bo_bc = singles.tile([128, 2048], BF16)
                nc.gpsimd.dma_start(
                    bo_bc,
                    bass.AP(tensor=bo[:].tensor, offset=0, ap=[[0, 128], [1, 2048]])
                )

            # Q in block-diag layout: qbd[32h+j, m, (b, 8h'+t)]
            qbd = singles.tile([128, 2, 1024], BF16)
            nc.vector.memset(qbd, 0.0)
            knew = singles.tile([128, 2, 256], BF16)  # [(h,j), m, tok]
            ones_col = singles.tile([128, 1], BF16)
            nc.vector.memset(ones_col, 1.0)
            ones_row = singles.tile([1, 128], F16)
            nc.vector.memset(ones_row, 1.0)
            recip_all = singles.tile([1, 1024], F16)  # 1/Z per (b, h, t)
            aoT = singles.tile([128, 2, 256], BF16)   # [64h'+dh, hp, tok]

            # persistent AV accumulator: [64h'+dh, (b, hp, h', t)]
            psav = ps_av.tile([128, 1024], F32)

            # ---- PE p-state warmup: keep the tensor engine continuously busy
            # while weights stream in, so QKV matmuls run at full clock.
            # Results land in psav rows 0-1, later cleared by AV's start=True.
            warm = singles.tile([128, 512], BF16)
            nc.vector.memset(warm, 0.0)
            for w in range(14):
                nc.tensor.matmul(psav[0:1, 0:512], ones_col, warm,
                                 start=True, stop=True)

            # ---- projections ----
            for m in range(2):
                psq = ps.tile([128, 512], F32, name=f"psq_{m}", tag="ps")[:, :256]
                psk = ps.tile([128, 512], F32, name=f"psk_{m}", tag="ps")[:, :256]
                for k in range(16):
                    st = dict(start=(k == 0), stop=(k == 15))
                    nc.tensor.matmul(
                        psq, wq_sb[:, k, 128 * m:128 * m + 128],
                        xT_sb[:, k, :], **st)
                for k in range(16):
                    st = dict(start=(k == 0), stop=(k == 15))
                    nc.tensor.matmul(
                        psk, wk_sb[:, k, 128 * m:128 * m + 128],
                        xT_sb[:, k, :], **st)
                # evac Q into block-diag (strided) + bias; psum rows 32h+j
                for h in range(4):
                    rows = slice(32 * h, 32 * h + 32)
                    out_ap = qbd[rows, m, :].rearrange("p (b w) -> p b w", w=32)[
                        :, :, 8 * h:8 * h + 8
                    ]
                    in_ap = psq[rows, :].rearrange("p (b t) -> p b t", t=8)
                    nc.scalar.activation(out_ap, in_ap, AF.Identity,
                                         bias=bq_sb[rows, m:m + 1], scale=1.0)
                nc.scalar.activation(knew[:, m, :], psk, AF.Identity,
                                     bias=bk_sb[:, m:m + 1], scale=1.0)

            vnew = []
            for m in range(2):
                psv = ps.tile([128, 512], F32, name=f"psv_{m}", tag="ps")[:, :256]
                for k in range(16):
                    st = dict(start=(k == 0), stop=(k == 15))
                    nc.tensor.matmul(
                        psv, xT_sb[:, k, 128 * m:128 * m + 128],
                        wv_sb[:, k, :], **st)
                vnew_sb = small.tile([128, 256], BF16, name=f"vnew_sb_{m}",
                                     tag="vnew", bufs=2)
                nc.vector.tensor_add(vnew_sb, psv, bv_bc)
                vnew.append(vnew_sb)

            # ---- hoisted batches' score chains, run up front ----
            attnT_h = {}
            for b in HOIST:
                attnT_h[b] = singles.tile([128, 8, 32], BF16, name=f"at_h{b}")
                nc.vector.tensor_copy(kt_h[b][:, :, :, 127],
                                      knew[:, :, 8 * b:8 * b + 8])
                sch = ps.tile([128, 512], F32, name=f"sc_h{b}", tag="ps")
                scTh = sch[:, :256].rearrange("p (i w) -> p i w", w=32)
                for i in range(8):
                    for m2 in range(2):
                        nc.tensor.matmul(
                            scTh[:, i, :], kt_h[b][:, m2, i, :],
                            qbd[:, m2, 32 * b:32 * b + 32],
                            start=(m2 == 0), stop=(m2 == 1))
                nc.scalar.activation(attnT_h[b], scTh, AF.Exp, scale=0.125)
                zth = ps.tile([128, 512], F32, name=f"zt_h{b}", tag="ps")
                for i in range(8):
                    nc.tensor.matmul(zth[0:1, 0:32], ones_col,
                                     attnT_h[b][:, i, :],
                                     start=(i == 0), stop=(i == 7))
                with nc.allow_low_precision(reason="1/Z in f16"):
                    nc.vector.reciprocal(recip_all[:, 32 * b:32 * b + 32],
                                         zth[0:1, 0:32])

            # ---- normalize + evacuate a quarter of psav into aoT ----
            def make_bc(q):
                bc = ps.tile([128, 512], F32, name=f"bc_{q}", tag="ps")[:, :256]
                nc.tensor.matmul(bc, ones_row,
                                 recip_all[:, 256 * q:256 * q + 256],
                                 start=True, stop=True)
                bc_sb = small.tile([128, 256], F16, name=f"bc_sb_{q}",
                                   tag="bcs", bufs=2)
                nc.scalar.copy(bc_sb, bc)
                return bc_sb

            def evac_range(q, bc_sb, j0, j1):
                # batches 8q+j0 .. 8q+j1 of quarter q
                nb = j1 - j0
                for hh in range(2):  # h' = partition half
                    rows = slice(64 * hh, 64 * hh + 64)
                    in0 = psav[rows,
                               256 * q + 32 * j0:256 * q + 32 * j1].rearrange(
                        "p (b hp hh t) -> p b hp hh t", b=nb, hp=2, t=8)[
                        :, :, :, hh, :]
                    in1 = bc_sb[rows, 32 * j0:32 * j1].rearrange(
                        "p (b hp hh t) -> p b hp hh t", b=nb, hp=2, t=8)[
                        :, :, :, hh, :]
                    out_ap = aoT[rows, :,
                                 64 * q + 8 * j0:64 * q + 8 * j1].rearrange(
                        "p a (b t) -> p b a t", t=8)
                    nc.vector.tensor_tensor(out_ap, in0, in1, ALU.mult)

            def evac_quarter(q):
                evac_range(q, make_bc(q), 0, 8)

            # ---- project one token-half (after its two quarters evac'd) ----
            def out_proj_half(half):
                psos = [ps.tile([128, 512], F32, name=f"pso_{half}_{ob}",
                                tag="ps") for ob in range(4)]
                for a in range(2):  # groups interleave across the 4 banks
                    for ob in range(4):
                        nc.tensor.matmul(
                            psos[ob], aoT[:, a, 128 * half:128 * half + 128],
                            wo_sb[:, a, 512 * ob:512 * ob + 512],
                            start=(a == 0), stop=(a == 1))
                for ob in range(4):
                    osb = small.tile([128, 512], BF16, name=f"osb_{half}_{ob}",
                                     tag="osb", bufs=8)
                    if bo_zero:
                        # bo == 0: plain psum evac, split DVE/ACT so the four
                        # chains drain two-wide in the tail
                        if ob % 2 == 0:
                            nc.vector.tensor_copy(osb, psos[ob])
                        else:
                            nc.scalar.copy(osb, psos[ob])
                    else:
                        nc.vector.tensor_add(osb, psos[ob],
                                             bo_bc[:, 512 * ob:512 * ob + 512])
                    if half == 0:  # Pool carries the vt stream: keep it clear
                        eng = [nc.sync, nc.sync, nc.sync, nc.scalar][ob]
                    else:
                        eng = [nc.sync, nc.gpsimd, nc.scalar, nc.sync][ob]
                    eng.dma_start(
                        out[128 * half:128 * half + 128, 512 * ob:512 * ob + 512],
                        osb)

            # ---- attention (per batch; last 4 scores ran up front) ----
            for b in range(B - len(HOIST)):
                kt = stream.tile([128, 2, 8, 128], BF16, name="kt", tag="kt",
                                 bufs=8)
                vt = stream.tile([128, 8, HC, DH], BF16, name="vt", tag="vt",
                                 bufs=8)
                nc.sync.dma_start(kt, kT[b])
                nc.gpsimd.dma_start(vt[0:P, :, :, :], v[b])
                # new V rows for this batch land on partition 127:
                # vt[127, i, h, d] = vnew[token 8b+i][64h+d]
                m, r0 = b // 16, 8 * (b % 16)
                nc.gpsimd.dma_start(vt[P:128, :, :, :], vnew[m][r0:r0 + 8, :])
                # new K columns: kt[:, m, i, 127] = knew[:, m, 8b+i]
                nc.vector.tensor_copy(kt[:, :, :, 127], knew[:, :, 8 * b:8 * b + 8])

                # scores^T: scT[p, i, (h,t)] = q . k(8p+i) / 8 (pre-scale in exp)
                sc = ps.tile([128, 512], F32, name="sc", tag="ps")
                scT = sc[:, :256].rearrange("p (i w) -> p i w", w=32)
                for i in range(8):
                    for m2 in range(2):
                        nc.tensor.matmul(
                            scT[:, i, :], kt[:, m2, i, :],
                            qbd[:, m2, 32 * b:32 * b + 32],
                            start=(m2 == 0), stop=(m2 == 1))

                attnT = stream.tile([128, 8, 32], BF16, name="attnT", tag="at",
                                    bufs=3)
                nc.scalar.activation(attnT, scT, AF.Exp, scale=0.125)

                # AV^T: psav[64h'+d, (b, hp, h', t)] += vt^T @ attnT
                for hp in range(2):
                    col = 32 * b + 16 * hp
                    for i in range(8):
                        nc.tensor.matmul(
                            psav[:, col:col + 16],
                            vt[:, i, 2 * hp:2 * hp + 2, :],
                            attnT[:, i, 16 * hp:16 * hp + 16],
                            start=(i == 0), stop=(i == 7))

                # Z[(h,t)] = sum_s exp: ones-matmul over partitions, acc over i
                zt = ps.tile([128, 512], F32, name="zt", tag="ps")
                for i in range(8):
                    nc.tensor.matmul(zt[0:1, 0:32], ones_col, attnT[:, i, :],
                                     start=(i == 0), stop=(i == 7))
                with nc.allow_low_precision(reason="1/Z in f16: 0.05% rel err"):
                    nc.vector.reciprocal(recip_all[:, 32 * b:32 * b + 32],
                                         zt[0:1, 0:32])

                if b % 8 == 7:
                    evac_quarter(b // 8)
                if b == 15:
                    out_proj_half(0)

            # ---- hoisted batches' tails: only AV depends on the vt stream.
            # bc for quarter 3 is ready before the stream drains; the psav
            # columns of b24..30 evacuate behind AV(30), so after the final
            # vt transfer only AV(31) + 32 evac columns + projection remain.
            bc3 = make_bc(3)
            for b in HOIST:
                vt = stream.tile([128, 8, HC, DH], BF16, name="vt", tag="vt",
                                 bufs=8)
                nc.gpsimd.dma_start(vt[0:P, :, :, :], v[b])
                m, r0 = b // 16, 8 * (b % 16)
                nc.gpsimd.dma_start(vt[P:128, :, :, :], vnew[m][r0:r0 + 8, :])
                for hp in range(2):
                    col = 32 * b + 16 * hp
                    for i in range(8):
                        nc.tensor.matmul(
                            psav[:, col:col + 16],
                            vt[:, i, 2 * hp:2 * hp + 2, :],
                            attnT_h[b][:, i, 16 * hp:16 * hp + 16],
                            start=(i == 0), stop=(i == 7))
                if b == B - 2:
                    evac_range(3, bc3, 0, 7)
            evac_range(3, bc3, 7, 8)
            out_proj_half(1)

    nc.finalize()
    return nc


def _prep_core(c, x_flat_T, cache_keys, cache_values, Wq, bq, Wk, bk, Wv, bv, Wo, bo):
    hs = slice(HC * c, HC * c + HC)
    qs = slice(QD * c, QD * c + QD)

    def perm_rows(W):
        # rows ordered (m, h, j): row 32h + j of tile m = W[64h + 32m + j]
        Ws = W[qs].reshape(HC, 2, 32, -1)              # [h, m, j, d]
        return Ws.transpose(1, 0, 2, 3).reshape(QD, -1)  # [(m,h,j), d]

    wq_p = perm_rows(Wq)
    wk_p = perm_rows(Wk)
    bq_p = np.ascontiguousarray(perm_rows(bq[:, None])[:, 0])
    bk_p = np.ascontiguousarray(perm_rows(bk[:, None])[:, 0])

    def as_tiles(WT):  # [D, 256] -> [128, 16, 256]
        return np.ascontiguousarray(
            WT.reshape(16, 128, QD).transpose(1, 0, 2)).astype(bfloat16)

    wqT = as_tiles(np.ascontiguousarray(wq_p.T))
    wkT = as_tiles(np.ascontiguousarray(wk_p.T))
    wvT = as_tiles(np.ascontiguousarray(Wv[qs].T))
    woT = np.ascontiguousarray(
        Wo[:, qs].T.reshape(2, 128, D).transpose(1, 0, 2)).astype(bfloat16)

    # kT[b, 32h+j, m, i, p] = K[b, h, 8p+i, 32m+j]; p=127 filled on device
    ck = cache_keys[:, hs]                        # [B, 4, 1016, 64]
    km = ck.reshape(B, HC, P, 8, 2, 32)           # [b, h, p, i, m, j]
    kT = np.zeros((B, HC, 32, 2, 8, 128), dtype=bfloat16)  # [b, h, j, m, i, p]
    kT[..., :P] = km.transpose(0, 1, 5, 4, 3, 2).astype(bfloat16)
    kT = kT.reshape(B, 128, 2, 8, 128)

    # v[b, p, i, h, d] = V[b, h, 8p+i, d]
    cv = cache_values[:, hs].reshape(B, HC, P, 8, DH)
    vv = np.ascontiguousarray(cv.transpose(0, 2, 3, 1, 4)).astype(bfloat16)

    return {
        "xT": x_flat_T.astype(bfloat16),
        "wqT": wqT, "wkT": wkT, "wvT": wvT, "woT": woT,
        "bq": bq_p.astype(np.float32), "bk": bk_p.astype(np.float32),
        "bv": np.ascontiguousarray(bv[qs]).astype(np.float32),
        "bo": bo.astype(bfloat16),
        "kT": kT,
        "v": vv,
    }


_NC_CACHE = {}


def kernel(x, cache_keys, cache_values, Wq, bq, Wk, bk, Wv, bv, Wo, bo):
    x = np.asarray(x, dtype=np.float32)
    cache_keys = np.asarray(cache_keys, dtype=np.float32)
    cache_values = np.asarray(cache_values, dtype=np.float32)
    Wq, Wk, Wv, Wo = (np.asarray(w, dtype=np.float32) for w in (Wq, Wk, Wv, Wo))
    bq, bk, bv, bo = (np.asarray(b_, dtype=np.float32) for b_ in (bq, bk, bv, bo))

    x_flat_T = np.ascontiguousarray(
        x.reshape(TOK, D).T.reshape(16, 128, TOK).transpose(1, 0, 2))  # [128,16,256]

    in_maps = [
        _prep_core(c, x_flat_T, cache_keys, cache_values,
                   Wq, bq, Wk, bk, Wv, bv, Wo, bo)
        for c in range(N_CORES)
    ]

    cfg = dict(CFG)
    cfg["bo_zero"] = not np.any(bo)
    key = tuple(sorted(cfg.items()))
    if key not in _NC_CACHE:
        _NC_CACHE[key] = build_nc(cfg)
    nc = _NC_CACHE[key]

    res = bass_utils.run_bass_kernel_spmd(nc, in_maps, core_ids=list(range(N_CORES)))
    out = np.zeros((TOK, D), dtype=np.float32)
    for r in res.results:
        out += r["out"].astype(np.float32)
    return out.reshape(B, T, D)


# revision 60
# speedup vs baseline: 1.1523x; 1.1523x over previous
"""Trainium2 Bass kernel for a single-layer MHA decode step with KV cache.

Problem (hardcoded from spec):
  x            [32, 8, 2048]      query tokens (B=32 batches x T=8 steps)
  cache_keys   [32, 32, 1016, 64] (B, H, S_cache, Dh)
  cache_values [32, 32, 1016, 64]
  Wq/Wk/Wv/Wo  [2048, 2048], biases [2048]
  out = MHA(x, cache) @ Wo.T + bo   -> [32, 8, 2048]

Sharding: tensor-parallel over heads. Each of the 8 cores handles 4 heads:
QKV projections for its head slice, attention over its KV-cache slice, and a
partial output projection (rank-256 slice of Wo). Host sums the 8 partials.

Design notes (v2 - transposed attention, bf16 streaming):
 - Everything DMA'd from DRAM is bf16: KV cache, weights, x, and the output
   partials. The kernel is HBM-bound on the KV cache (~33 MB/core in bf16),
   so halving wire bytes halves runtime; bf16 keeps rel-err ~5e-3 under the
   2e-2 gate. Cost-model time 120.9 us with the DMA device 92% busy at its
   modeled 360 GB/s (111.2 us of transfers = the bf16 byte floor).
 - Scores are computed TRANSPOSED: scT[s, (h,t)] per batch, with the key
   tile as the stationary matmul operand and the block-diagonal q as the
   moving operand (free dim 32).  s lives on partitions as s = 8p + i with
   i = 0..7 the free-dim chunk; p = 127 holds the 8 freshly projected keys
   (token 8b+i at chunk i), copied into the key tile on device.
 - Softmax: no max subtraction (scores are O(1), exp is safe in f32/bf16).
   exp runs on ACT into bf16 attnT; the normalizer Z per query is a
   ones-vector matmul over partitions; normalization is deferred to the
   psav evacuation (out = (1/Z) * sum exp*v factorizes).
 - AV is also transposed: out[dh, t] with v stationary, attn moving
   (free dim 16 covering a pair of heads), accumulated over the 8 s-chunks
   into one persistent [128, 1024] PSUM region laid out as aoT.
 - Output projection reads aoT directly; partials are written as bf16 and
   summed on host. It runs per token-half: half 0 mid-stream (hidden),
   half 1 in the tail.
 - Issue queues: kt on SP (HWDGE), vt + vnew on Pool (SWDGE, no HWDGE
   contention), exp/evac on ACT. Each DMA issue holds its queue's SEQ
   through desc-gen (~1.2 us), so one queue cannot feed both big streams.
 - Tail: the last 4 batches' key tiles load up front and their score
   chains run early, so after the final vt transfer only AV + a 32-column
   evac + the half-1 projection remain. 14 dummy matmuls at t=0 warm the
   PE p-state so QKV runs at full clock.
"""

import numpy as np
from ml_dtypes import bfloat16, float8_e3m4

import concourse.bass as bass
import concourse.mybir as mybir
import concourse.tile as tile
from concourse import bacc
from concourse import bass_utils

F32 = mybir.dt.float32
F16 = mybir.dt.float16
BF16 = mybir.dt.bfloat16
F8E3 = mybir.dt.float8e3

B, T, D = 32, 8, 2048
H, DH = 32, 64
S_CACHE, S = 1016, 1024
N_CORES = 8
HC = H // N_CORES          # heads per core = 4
TOK = B * T                # 256
QD = HC * DH               # 256 per-core qkv dims
P = 127                    # s-rows per chunk from the cache (1016 = 8*127)

AF = mybir.ActivationFunctionType
ALU = mybir.AluOpType
AX = mybir.AxisListType

CFG = {"dtype": "bf16", "bo_zero": True}


def build_nc(cfg=CFG):
    bo_zero = cfg.get("bo_zero", False)
    nc = bacc.Bacc(None, target_bir_lowering=False)

    xT = nc.dram_tensor("xT", [128, 16, 256], BF16, kind="ExternalInput")
    wqT = nc.dram_tensor("wqT", [128, 16, 256], BF16, kind="ExternalInput")
    wkT = nc.dram_tensor("wkT", [128, 16, 256], BF16, kind="ExternalInput")
    wvT = nc.dram_tensor("wvT", [128, 16, 256], BF16, kind="ExternalInput")
    woT = nc.dram_tensor("woT", [128, 2, 2048], BF16, kind="ExternalInput")
    bq = nc.dram_tensor("bq", [256], F32, kind="ExternalInput")
    bk = nc.dram_tensor("bk", [256], F32, kind="ExternalInput")
    bv = nc.dram_tensor("bv", [256], F32, kind="ExternalInput")
    bo = nc.dram_tensor("bo", [2048], BF16, kind="ExternalInput")
    # kT[b, (h,j), m, i, p]: keys with s = 8p+i on tile axes; p=127 is filled
    # on device with the new key of token 8b+i.
    kT = nc.dram_tensor("kT", [B, 128, 2, 8, 128], BF16, kind="ExternalInput")
    # v[b, p, i, h, dh] = cache_values[b, h, 8p+i, dh]
    v = nc.dram_tensor("v", [B, P, 8, HC, DH], F8E3, kind="ExternalInput")
    out = nc.dram_tensor("out", [TOK, D], BF16, kind="ExternalOutput")

    with tile.TileContext(nc) as tc:
        with (
            tc.tile_pool(name="singles", bufs=1) as singles,
            tc.tile_pool(name="stream", bufs=8) as stream,
            tc.tile_pool(name="small", bufs=8) as small,
            tc.tile_pool(name="ps", bufs=6, space="PSUM") as ps,
            tc.tile_pool(name="ps_av", bufs=1, space="PSUM") as ps_av,
        ):
            # ---- persistent tiles ----
            xT_sb = singles.tile([128, 16, 256], BF16)
            wq_sb = singles.tile([128, 16, 256], BF16)
            wk_sb = singles.tile([128, 16, 256], BF16)
            wv_sb = singles.tile([128, 16, 256], BF16)
            wo_sb = singles.tile([128, 2, 2048], BF16)
            # weight loads split across issue queues: SP and Act alternate so
            # the shared HWDGE stage doesn't serialize one queue's prefetch
            nc.sync.dma_start(xT_sb, xT[:, :, :])
            nc.scalar.dma_start(wq_sb, wqT[:, :, :])
            nc.sync.dma_start(wk_sb, wkT[:, :, :])
            nc.scalar.dma_start(wv_sb, wvT[:, :, :])
            bq_sb = singles.tile([128, 2], F32)
            bk_sb = singles.tile([128, 2], F32)
            nc.gpsimd.dma_start(bq_sb, bq[:].rearrange("(m p) -> p m", p=128))
            nc.gpsimd.dma_start(bk_sb, bk[:].rearrange("(m p) -> p m", p=128))
            bv_bc = singles.tile([128, 256], F32)
            nc.gpsimd.dma_start(
                bv_bc, bass.AP(tensor=bv[:].tensor, offset=0, ap=[[0, 128], [1, 256]])
            )
            nc.gpsimd.dma_start(wo_sb, woT[:, :, :])
            # the LAST batches' keys load up front: their scores/exp/Z/recip
            # run early, so the tail after the final vt transfers is just
            # AV + evac + projection (no softmax chain on the critical path)
            HOIST = [28, 29, 30, 31]
            kt_h = {}
            for b in HOIST:
                kt_h[b] = singles.tile([128, 2, 8, 128], BF16, name=f"kt_h{b}")
                nc.sync.dma_start(kt_h[b], kT[b])
            if not bo_zero:
                bo_bc = singles.tile([128, 2048], BF16)
                nc.gpsimd.dma_start(
                    bo_bc,
                    bass.AP(tensor=bo[:].tensor, offset=0, ap=[[0, 128], [1, 2048]])
                )

            # Q in block-diag layout: qbd[32h+j, m, (b, 8h'+t)]
            qbd = singles.tile([128, 2, 1024], BF16)
            nc.vector.memset(qbd, 0.0)
            knew = singles.tile([128, 2, 256], BF16)  # [(h,j), m, tok]
            ones_col = singles.tile([128, 1], BF16)
            nc.vector.memset(ones_col, 1.0)
            ones_row = singles.tile([1, 128], F16)
            nc.vector.memset(ones_row, 1.0)
            recip_all = singles.tile([1, 1024], F16)  # 1/Z per (b, h, t)
            aoT = singles.tile([128, 2, 256], BF16)   # [64h'+dh, hp, tok]

            # persistent AV accumulator: [64h'+dh, (b, hp, h', t)]
            psav = ps_av.tile([128, 1024], F32)

            # ---- PE p-state warmup: keep the tensor engine continuously busy
            # while weights stream in, so QKV matmuls run at full clock.
            # Results land in psav rows 0-1, later cleared by AV's start=True.
            warm = singles.tile([128, 512], BF16)
            nc.vector.memset(warm, 0.0)
            for w in range(14):
                nc.tensor.matmul(psav[0:1, 0:512], ones_col, warm,
                                 start=True, stop=True)

            # ---- projections ----
            for m in range(2):
                psq = ps.tile([128, 512], F32, name=f"psq_{m}", tag="ps")[:, :256]
                psk = ps.tile([128, 512], F32, name=f"psk_{m}", tag="ps")[:, :256]
                for k in range(16):
                    st = dict(start=(k == 0), stop=(k == 15))
                    nc.tensor.matmul(
                        psq, wq_sb[:, k, 128 * m:128 * m + 128],
                        xT_sb[:, k, :], **st)
                for k in range(16):
                    st = dict(start=(k == 0), stop=(k == 15))
                    nc.tensor.matmul(
                        psk, wk_sb[:, k, 128 * m:128 * m + 128],
                        xT_sb[:, k, :], **st)
                # evac Q into block-diag (strided) + bias; psum rows 32h+j
                for h in range(4):
                    rows = slice(32 * h, 32 * h + 32)
                    out_ap = qbd[rows, m, :].rearrange("p (b w) -> p b w", w=32)[
                        :, :, 8 * h:8 * h + 8
                    ]
                    in_ap = psq[rows, :].rearrange("p (b t) -> p b t", t=8)
                    nc.scalar.activation(out_ap, in_ap, AF.Identity,
                                         bias=bq_sb[rows, m:m + 1], scale=1.0)
                nc.scalar.activation(knew[:, m, :], psk, AF.Identity,
                                     bias=bk_sb[:, m:m + 1], scale=1.0)

            vnew = []
            for m in range(2):
                psv = ps.tile([128, 512], F32, name=f"psv_{m}", tag="ps")[:, :256]
                for k in range(16):
                    st = dict(start=(k == 0), stop=(k == 15))
                    nc.tensor.matmul(
                        psv, xT_sb[:, k, 128 * m:128 * m + 128],
                        wv_sb[:, k, :], **st)
                vnew_sb = small.tile([128, 256], F8E3, name=f"vnew_sb_{m}",
                                     tag="vnew", bufs=2)
                with nc.allow_low_precision(reason="V stream is fp8 e3m4"):
                    nc.vector.tensor_add(vnew_sb, psv, bv_bc)
                vnew.append(vnew_sb)

            # ---- hoisted batches' score chains, run up front ----
            attnT_h = {}
            for b in HOIST:
                attnT_h[b] = singles.tile([128, 8, 32], BF16, name=f"at_h{b}")
                nc.vector.tensor_copy(kt_h[b][:, :, :, 127],
                                      knew[:, :, 8 * b:8 * b + 8])
                sch = ps.tile([128, 512], F32, name=f"sc_h{b}", tag="ps")
                scTh = sch[:, :256].rearrange("p (i w) -> p i w", w=32)
                for i in range(8):
                    for m2 in range(2):
                        nc.tensor.matmul(
                            scTh[:, i, :], kt_h[b][:, m2, i, :],
                            qbd[:, m2, 32 * b:32 * b + 32],
                            start=(m2 == 0), stop=(m2 == 1))
                nc.scalar.activation(attnT_h[b], scTh, AF.Exp, scale=0.125)
                zth = ps.tile([128, 512], F32, name=f"zt_h{b}", tag="ps")
                for i in range(8):
                    nc.tensor.matmul(zth[0:1, 0:32], ones_col,
                                     attnT_h[b][:, i, :],
                                     start=(i == 0), stop=(i == 7))
                with nc.allow_low_precision(reason="1/Z in f16"):
                    nc.vector.reciprocal(recip_all[:, 32 * b:32 * b + 32],
                                         zth[0:1, 0:32])

            # ---- normalize + evacuate a quarter of psav into aoT ----
            def make_bc(q):
                bc = ps.tile([128, 512], F32, name=f"bc_{q}", tag="ps")[:, :256]
                nc.tensor.matmul(bc, ones_row,
                                 recip_all[:, 256 * q:256 * q + 256],
                                 start=True, stop=True)
                bc_sb = small.tile([128, 256], F16, name=f"bc_sb_{q}",
                                   tag="bcs", bufs=2)
                nc.scalar.copy(bc_sb, bc)
                return bc_sb

            def evac_range(q, bc_sb, j0, j1):
                # batches 8q+j0 .. 8q+j1 of quarter q
                nb = j1 - j0
                for hh in range(2):  # h' = partition half
                    rows = slice(64 * hh, 64 * hh + 64)
                    in0 = psav[rows,
                               256 * q + 32 * j0:256 * q + 32 * j1].rearrange(
                        "p (b hp hh t) -> p b hp hh t", b=nb, hp=2, t=8)[
                        :, :, :, hh, :]
                    in1 = bc_sb[rows, 32 * j0:32 * j1].rearrange(
                        "p (b hp hh t) -> p b hp hh t", b=nb, hp=2, t=8)[
                        :, :, :, hh, :]
                    out_ap = aoT[rows, :,
                                 64 * q + 8 * j0:64 * q + 8 * j1].rearrange(
                        "p a (b t) -> p b a t", t=8)
                    nc.vector.tensor_tensor(out_ap, in0, in1, ALU.mult)

            def evac_quarter(q):
                evac_range(q, make_bc(q), 0, 8)

            # ---- project one token-half (after its two quarters evac'd) ----
            def out_proj_half(half):
                psos = [ps.tile([128, 512], F32, name=f"pso_{half}_{ob}",
                                tag="ps") for ob in range(4)]
                for a in range(2):  # groups interleave across the 4 banks
                    for ob in range(4):
                        nc.tensor.matmul(
                            psos[ob], aoT[:, a, 128 * half:128 * half + 128],
                            wo_sb[:, a, 512 * ob:512 * ob + 512],
                            start=(a == 0), stop=(a == 1))
                for ob in range(4):
                    osb = small.tile([128, 512], BF16, name=f"osb_{half}_{ob}",
                                     tag="osb", bufs=8)
                    if bo_zero:
                        # bo == 0: plain psum evac, split DVE/ACT so the four
                        # chains drain two-wide in the tail
                        if ob % 2 == 0:
                            nc.vector.tensor_copy(osb, psos[ob])
                        else:
                            nc.scalar.copy(osb, psos[ob])
                    else:
                        nc.vector.tensor_add(osb, psos[ob],
                                             bo_bc[:, 512 * ob:512 * ob + 512])
                    if half == 0:  # Pool carries the vt stream: keep it clear
                        eng = [nc.sync, nc.sync, nc.sync, nc.scalar][ob]
                    else:
                        eng = [nc.sync, nc.gpsimd, nc.scalar, nc.sync][ob]
                    eng.dma_start(
                        out[128 * half:128 * half + 128, 512 * ob:512 * ob + 512],
                        osb)

            # ---- attention (per batch; last 4 scores ran up front) ----
            for b in range(B - len(HOIST)):
                kt = stream.tile([128, 2, 8, 128], BF16, name="kt", tag="kt",
                                 bufs=8)
                vt = stream.tile([128, 8, HC, DH], F8E3, name="vt", tag="vt",
                                 bufs=8)
                nc.gpsimd.dma_start(kt, kT[b])
                nc.sync.dma_start(vt[0:P, :, :, :], v[b])
                # new V rows for this batch land on partition 127:
                # vt[127, i, h, d] = vnew[token 8b+i][64h+d]
                m, r0 = b // 16, 8 * (b % 16)
                nc.gpsimd.dma_start(vt[P:128, :, :, :], vnew[m][r0:r0 + 8, :])
                # new K columns: kt[:, m, i, 127] = knew[:, m, 8b+i]
                nc.vector.tensor_copy(kt[:, :, :, 127], knew[:, :, 8 * b:8 * b + 8])

                # scores^T: scT[p, i, (h,t)] = q . k(8p+i) / 8 (pre-scale in exp)
                sc = ps.tile([128, 512], F32, name="sc", tag="ps")
                scT = sc[:, :256].rearrange("p (i w) -> p i w", w=32)
                for i in range(8):
                    for m2 in range(2):
                        nc.tensor.matmul(
                            scT[:, i, :], kt[:, m2, i, :],
                            qbd[:, m2, 32 * b:32 * b + 32],
                            start=(m2 == 0), stop=(m2 == 1))

                attnT = stream.tile([128, 8, 32], BF16, name="attnT", tag="at",
                                    bufs=3)
                nc.scalar.activation(attnT, scT, AF.Exp, scale=0.125)

                # AV^T: psav[64h'+d, (b, hp, h', t)] += vt^T @ attnT
                for hp in range(2):
                    col = 32 * b + 16 * hp
                    for i in range(8):
                        nc.tensor.matmul(
                            psav[:, col:col + 16],
                            vt[:, i, 2 * hp:2 * hp + 2, :],
                            attnT[:, i, 16 * hp:16 * hp + 16],
                            start=(i == 0), stop=(i == 7))

                # Z[(h,t)] = sum_s exp: ones-matmul over partitions, acc over i
                zt = ps.tile([128, 512], F32, name="zt", tag="ps")
                for i in range(8):
                    nc.tensor.matmul(zt[0:1, 0:32], ones_col, attnT[:, i, :],
                                     start=(i == 0), stop=(i == 7))
                with nc.allow_low_precision(reason="1/Z in f16: 0.05% rel err"):
                    nc.vector.reciprocal(recip_all[:, 32 * b:32 * b + 32],
                                         zt[0:1, 0:32])

                if b % 8 == 7:
                    evac_quarter(b // 8)
                if b == 15:
                    out_proj_half(0)

            # ---- hoisted batches' tails: only AV depends on the vt stream.
            # bc for quarter 3 is ready before the stream drains; the psav
            # columns of b24..30 evacuate behind AV(30), so after the final
            # vt transfer only AV(31) + 32 evac columns + projection remain.
            bc3 = make_bc(3)
            for b in HOIST:
                vt = stream.tile([128, 8, HC, DH], F8E3, name="vt", tag="vt",
                                 bufs=8)
                nc.gpsimd.dma_start(vt[0:P, :, :, :], v[b])
                m, r0 = b // 16, 8 * (b % 16)
                nc.gpsimd.dma_start(vt[P:128, :, :, :], vnew[m][r0:r0 + 8, :])
                for hp in range(2):
                    col = 32 * b + 16 * hp
                    for i in range(8):
                        nc.tensor.matmul(
                            psav[:, col:col + 16],
                            vt[:, i, 2 * hp:2 * hp + 2, :],
                            attnT_h[b][:, i, 16 * hp:16 * hp + 16],
                            start=(i == 0), stop=(i == 7))
                if b == B - 2:
                    evac_range(3, bc3, 0, 7)
            evac_range(3, bc3, 7, 8)
            out_proj_half(1)

    nc.finalize()
    return nc


def _prep_core(c, x_flat_T, cache_keys, cache_values, Wq, bq, Wk, bk, Wv, bv, Wo, bo):
    hs = slice(HC * c, HC * c + HC)
    qs = slice(QD * c, QD * c + QD)

    def perm_rows(W):
        # rows ordered (m, h, j): row 32h + j of tile m = W[64h + 32m + j]
        Ws = W[qs].reshape(HC, 2, 32, -1)              # [h, m, j, d]
        return Ws.transpose(1, 0, 2, 3).reshape(QD, -1)  # [(m,h,j), d]

    wq_p = perm_rows(Wq)
    wk_p = perm_rows(Wk)
    bq_p = np.ascontiguousarray(perm_rows(bq[:, None])[:, 0])
    bk_p = np.ascontiguousarray(perm_rows(bk[:, None])[:, 0])

    def as_tiles(WT):  # [D, 256] -> [128, 16, 256]
        return np.ascontiguousarray(
            WT.reshape(16, 128, QD).transpose(1, 0, 2)).astype(bfloat16)

    wqT = as_tiles(np.ascontiguousarray(wq_p.T))
    wkT = as_tiles(np.ascontiguousarray(wk_p.T))
    wvT = as_tiles(np.ascontiguousarray(Wv[qs].T))
    woT = np.ascontiguousarray(
        Wo[:, qs].T.reshape(2, 128, D).transpose(1, 0, 2)).astype(bfloat16)

    # kT[b, 32h+j, m, i, p] = K[b, h, 8p+i, 32m+j]; p=127 filled on device
    ck = cache_keys[:, hs]                        # [B, 4, 1016, 64]
    km = ck.reshape(B, HC, P, 8, 2, 32)           # [b, h, p, i, m, j]
    kT = np.zeros((B, HC, 32, 2, 8, 128), dtype=bfloat16)  # [b, h, j, m, i, p]
    kT[..., :P] = km.transpose(0, 1, 5, 4, 3, 2).astype(bfloat16)
    kT = kT.reshape(B, 128, 2, 8, 128)

    # v[b, p, i, h, d] = V[b, h, 8p+i, d]
    cv = cache_values[:, hs].reshape(B, HC, P, 8, DH)
    vv = np.ascontiguousarray(cv.transpose(0, 2, 3, 1, 4)).astype(float8_e3m4)

    return {
        "xT": x_flat_T.astype(bfloat16),
        "wqT": wqT, "wkT": wkT, "wvT": wvT, "woT": woT,
        "bq": bq_p.astype(np.float32), "bk": bk_p.astype(np.float32),
        "bv": np.ascontiguousarray(bv[qs]).astype(np.float32),
        "bo": bo.astype(bfloat16),
        "kT": kT,
        "v": vv,
    }


_NC_CACHE = {}


def kernel(x, cache_keys, cache_values, Wq, bq, Wk, bk, Wv, bv, Wo, bo):
    x = np.asarray(x, dtype=np.float32)
    cache_keys = np.asarray(cache_keys, dtype=np.float32)
    cache_values = np.asarray(cache_values, dtype=np.float32)
    Wq, Wk, Wv, Wo = (np.asarray(w, dtype=np.float32) for w in (Wq, Wk, Wv, Wo))
    bq, bk, bv, bo = (np.asarray(b_, dtype=np.float32) for b_ in (bq, bk, bv, bo))

    x_flat_T = np.ascontiguousarray(
        x.reshape(TOK, D).T.reshape(16, 128, TOK).transpose(1, 0, 2))  # [128,16,256]

    in_maps = [
        _prep_core(c, x_flat_T, cache_keys, cache_values,
                   Wq, bq, Wk, bk, Wv, bv, Wo, bo)
        for c in range(N_CORES)
    ]

    cfg = dict(CFG)
    cfg["bo_zero"] = not np.any(bo)
    key = tuple(sorted(cfg.items()))
    if key not in _NC_CACHE:
        _NC_CACHE[key] = build_nc(cfg)
    nc = _NC_CACHE[key]

    res = bass_utils.run_bass_kernel_spmd(nc, in_maps, core_ids=list(range(N_CORES)))
    out = np.zeros((TOK, D), dtype=np.float32)
    for r in res.results:
        out += r["out"].astype(np.float32)
    return out.reshape(B, T, D)


# revision 69
# speedup vs baseline: 1.1880x; 1.0310x over previous
"""Trainium2 Bass kernel for a single-layer MHA decode step with KV cache.

Problem (hardcoded from spec):
  x            [32, 8, 2048]      query tokens (B=32 batches x T=8 steps)
  cache_keys   [32, 32, 1016, 64] (B, H, S_cache, Dh)
  cache_values [32, 32, 1016, 64]
  Wq/Wk/Wv/Wo  [2048, 2048], biases [2048]
  out = MHA(x, cache) @ Wo.T + bo   -> [32, 8, 2048]

Sharding: tensor-parallel over heads. Each of the 8 cores handles 4 heads:
QKV projections for its head slice, attention over its KV-cache slice, and a
partial output projection (rank-256 slice of Wo). Host sums the 8 partials.

Design notes (v2 - transposed attention, bf16 streaming):
 - Everything DMA'd from DRAM is bf16: KV cache, weights, x, and the output
   partials. The kernel is HBM-bound on the KV cache (~33 MB/core in bf16),
   so halving wire bytes halves runtime; bf16 keeps rel-err ~5e-3 under the
   2e-2 gate. Cost-model time 120.9 us with the DMA device 92% busy at its
   modeled 360 GB/s (111.2 us of transfers = the bf16 byte floor).
 - Scores are computed TRANSPOSED: scT[s, (h,t)] per batch, with the key
   tile as the stationary matmul operand and the block-diagonal q as the
   moving operand (free dim 32).  s lives on partitions as s = 8p + i with
   i = 0..7 the free-dim chunk; p = 127 holds the 8 freshly projected keys
   (token 8b+i at chunk i), copied into the key tile on device.
 - Softmax: no max subtraction (scores are O(1), exp is safe in f32/bf16).
   exp runs on ACT into bf16 attnT; the normalizer Z per query is a
   ones-vector matmul over partitions; normalization is deferred to the
   psav evacuation (out = (1/Z) * sum exp*v factorizes).
 - AV is also transposed: out[dh, t] with v stationary, attn moving
   (free dim 16 covering a pair of heads), accumulated over the 8 s-chunks
   into one persistent [128, 1024] PSUM region laid out as aoT.
 - Output projection reads aoT directly; partials are written as bf16 and
   summed on host. It runs per token-half: half 0 mid-stream (hidden),
   half 1 in the tail.
 - Issue queues: kt on SP (HWDGE), vt + vnew on Pool (SWDGE, no HWDGE
   contention), exp/evac on ACT. Each DMA issue holds its queue's SEQ
   through desc-gen (~1.2 us), so one queue cannot feed both big streams.
 - Tail: the last 4 batches' key tiles load up front and their score
   chains run early, so after the final vt transfer only AV + a 32-column
   evac + the half-1 projection remain. 14 dummy matmuls at t=0 warm the
   PE p-state so QKV runs at full clock.
"""

import numpy as np
from ml_dtypes import bfloat16, float8_e3m4

import concourse.bass as bass
import concourse.mybir as mybir
import concourse.tile as tile
from concourse import bacc
from concourse import bass_utils

F32 = mybir.dt.float32
F16 = mybir.dt.float16
BF16 = mybir.dt.bfloat16
F8E3 = mybir.dt.float8e3

B, T, D = 32, 8, 2048
H, DH = 32, 64
S_CACHE, S = 1016, 1024
N_CORES = 8
HC = H // N_CORES          # heads per core = 4
TOK = B * T                # 256
QD = HC * DH               # 256 per-core qkv dims
P = 127                    # s-rows per chunk from the cache (1016 = 8*127)

AF = mybir.ActivationFunctionType
ALU = mybir.AluOpType
AX = mybir.AxisListType

CFG = {"dtype": "bf16", "bo_zero": True}


def build_nc(cfg=CFG):
    bo_zero = cfg.get("bo_zero", False)
    nc = bacc.Bacc(None, target_bir_lowering=False)

    xT = nc.dram_tensor("xT", [128, 16, 256], BF16, kind="ExternalInput")
    wqT = nc.dram_tensor("wqT", [128, 16, 256], BF16, kind="ExternalInput")
    wkT = nc.dram_tensor("wkT", [128, 16, 256], BF16, kind="ExternalInput")
    wvT = nc.dram_tensor("wvT", [128, 16, 256], BF16, kind="ExternalInput")
    woT = nc.dram_tensor("woT", [128, 2, 2048], BF16, kind="ExternalInput")
    bq = nc.dram_tensor("bq", [256], F32, kind="ExternalInput")
    bk = nc.dram_tensor("bk", [256], F32, kind="ExternalInput")
    bv = nc.dram_tensor("bv", [256], F32, kind="ExternalInput")
    bo = nc.dram_tensor("bo", [2048], BF16, kind="ExternalInput")
    # kT[b, (h,j), m, i, p]: keys with s = 8p+i on tile axes; p=127 is filled
    # on device with the new key of token 8b+i.
    kT = nc.dram_tensor("kT", [B, 128, 2, 8, 128], BF16, kind="ExternalInput")
    # v[b, p, i, h, dh] = cache_values[b, h, 8p+i, dh]
    v = nc.dram_tensor("v", [B, P, 8, HC, DH], F8E3, kind="ExternalInput")
    out = nc.dram_tensor("out", [TOK, D], BF16, kind="ExternalOutput")

    with tile.TileContext(nc) as tc:
        with (
            tc.tile_pool(name="singles", bufs=1) as singles,
            tc.tile_pool(name="stream", bufs=8) as stream,
            tc.tile_pool(name="small", bufs=8) as small,
            tc.tile_pool(name="ps", bufs=6, space="PSUM") as ps,
            tc.tile_pool(name="ps_av", bufs=1, space="PSUM") as ps_av,
        ):
            # ---- persistent tiles ----
            xT_sb = singles.tile([128, 16, 256], BF16)
            wq_sb = singles.tile([128, 16, 256], BF16)
            wk_sb = singles.tile([128, 16, 256], BF16)
            wv_sb = singles.tile([128, 16, 256], BF16)
            wo_sb = singles.tile([128, 2, 2048], BF16)
            # weight loads split across issue queues: SP and Act alternate so
            # the shared HWDGE stage doesn't serialize one queue's prefetch
            nc.sync.dma_start(xT_sb, xT[:, :, :])
            nc.scalar.dma_start(wq_sb, wqT[:, :, :])
            nc.sync.dma_start(wk_sb, wkT[:, :, :])
            nc.scalar.dma_start(wv_sb, wvT[:, :, :])
            bq_sb = singles.tile([128, 2], F32)
            bk_sb = singles.tile([128, 2], F32)
            nc.gpsimd.dma_start(bq_sb, bq[:].rearrange("(m p) -> p m", p=128))
            nc.gpsimd.dma_start(bk_sb, bk[:].rearrange("(m p) -> p m", p=128))
            bv_bc = singles.tile([128, 256], F32)
            nc.gpsimd.dma_start(
                bv_bc, bass.AP(tensor=bv[:].tensor, offset=0, ap=[[0, 128], [1, 256]])
            )
            nc.gpsimd.dma_start(wo_sb, woT[:, :, :])
            # the LAST batches' keys load up front: their scores/exp/Z/recip
            # run early, so the tail after the final vt transfers is just
            # AV + evac + projection (no softmax chain on the critical path)
            HOIST = [28, 29, 30, 31]
            kt_h = {}
            for b in HOIST:
                kt_h[b] = singles.tile([128, 2, 8, 128], BF16, name=f"kt_h{b}")
                nc.sync.dma_start(kt_h[b], kT[b])
            if not bo_zero:
                bo_bc = singles.tile([128, 2048], BF16)
                nc.gpsimd.dma_start(
                    bo_bc,
                    bass.AP(tensor=bo[:].tensor, offset=0, ap=[[0, 128], [1, 2048]])
                )

            # Q in block-diag layout: qbd[32h+j, m, (b, 8h'+t)]
            qbd = singles.tile([128, 2, 1024], BF16)
            nc.vector.memset(qbd, 0.0)
            knew = singles.tile([128, 2, 256], BF16)  # [(h,j), m, tok]
            ones_col = singles.tile([128, 1], BF16)
            nc.vector.memset(ones_col, 1.0)
            ones_row = singles.tile([1, 128], F16)
            nc.vector.memset(ones_row, 1.0)
            recip_all = singles.tile([1, 1024], F16)  # 1/Z per (b, h, t)
            aoT = singles.tile([128, 2, 256], BF16)   # [64h'+dh, hp, tok]

            # persistent AV accumulator: [64h'+dh, (b, hp, h', t)]
            psav = ps_av.tile([128, 1024], F32)

            # ---- PE p-state warmup: keep the tensor engine continuously busy
            # while weights stream in, so QKV matmuls run at full clock.
            # Results land in psav rows 0-1, later cleared by AV's start=True.
            warm = singles.tile([128, 512], BF16)
            nc.vector.memset(warm, 0.0)
            for w in range(14):
                nc.tensor.matmul(psav[0:1, 0:512], ones_col, warm,
                                 start=True, stop=True)

            # ---- projections ----
            for m in range(2):
                psq = ps.tile([128, 512], F32, name=f"psq_{m}", tag="ps")[:, :256]
                psk = ps.tile([128, 512], F32, name=f"psk_{m}", tag="ps")[:, :256]
                for k in range(16):
                    st = dict(start=(k == 0), stop=(k == 15))
                    nc.tensor.matmul(
                        psq, wq_sb[:, k, 128 * m:128 * m + 128],
                        xT_sb[:, k, :], **st)
                for k in range(16):
                    st = dict(start=(k == 0), stop=(k == 15))
                    nc.tensor.matmul(
                        psk, wk_sb[:, k, 128 * m:128 * m + 128],
                        xT_sb[:, k, :], **st)
                # evac Q into block-diag (strided) + bias; psum rows 32h+j
                for h in range(4):
                    rows = slice(32 * h, 32 * h + 32)
                    out_ap = qbd[rows, m, :].rearrange("p (b w) -> p b w", w=32)[
                        :, :, 8 * h:8 * h + 8
                    ]
                    in_ap = psq[rows, :].rearrange("p (b t) -> p b t", t=8)
                    nc.scalar.activation(out_ap, in_ap, AF.Identity,
                                         bias=bq_sb[rows, m:m + 1], scale=1.0)
                nc.scalar.activation(knew[:, m, :], psk, AF.Identity,
                                     bias=bk_sb[:, m:m + 1], scale=1.0)

            vnew = []
            for m in range(2):
                psv = ps.tile([128, 512], F32, name=f"psv_{m}", tag="ps")[:, :256]
                for k in range(16):
                    st = dict(start=(k == 0), stop=(k == 15))
                    nc.tensor.matmul(
                        psv, xT_sb[:, k, 128 * m:128 * m + 128],
                        wv_sb[:, k, :], **st)
                vnew_sb = small.tile([128, 256], F8E3, name=f"vnew_sb_{m}",
                                     tag="vnew", bufs=2)
                with nc.allow_low_precision(reason="V stream is fp8 e3m4"):
                    nc.vector.tensor_add(vnew_sb, psv, bv_bc)
                vnew.append(vnew_sb)

            # ---- hoisted batches' score chains, run up front ----
            attnT_h = {}
            for b in HOIST:
                attnT_h[b] = singles.tile([128, 8, 32], BF16, name=f"at_h{b}")
                nc.vector.tensor_copy(kt_h[b][:, :, :, 127],
                                      knew[:, :, 8 * b:8 * b + 8])
                sch = ps.tile([128, 512], F32, name=f"sc_h{b}", tag="ps")
                scTh = sch[:, :256].rearrange("p (i w) -> p i w", w=32)
                for i in range(8):
                    for m2 in range(2):
                        nc.tensor.matmul(
                            scTh[:, i, :], kt_h[b][:, m2, i, :],
                            qbd[:, m2, 32 * b:32 * b + 32],
                            start=(m2 == 0), stop=(m2 == 1))
                nc.scalar.activation(attnT_h[b], scTh, AF.Exp, scale=0.125)
                zth = ps.tile([128, 512], F32, name=f"zt_h{b}", tag="ps")
                for i in range(8):
                    nc.tensor.matmul(zth[0:1, 0:32], ones_col,
                                     attnT_h[b][:, i, :],
                                     start=(i == 0), stop=(i == 7))
                with nc.allow_low_precision(reason="1/Z in f16"):
                    nc.vector.reciprocal(recip_all[:, 32 * b:32 * b + 32],
                                         zth[0:1, 0:32])

            # ---- normalize + evacuate a quarter of psav into aoT ----
            def make_bc(q):
                bc = ps.tile([128, 512], F32, name=f"bc_{q}", tag="ps")[:, :256]
                nc.tensor.matmul(bc, ones_row,
                                 recip_all[:, 256 * q:256 * q + 256],
                                 start=True, stop=True)
                bc_sb = small.tile([128, 256], F16, name=f"bc_sb_{q}",
                                   tag="bcs", bufs=2)
                nc.scalar.copy(bc_sb, bc)
                return bc_sb

            def evac_range(q, bc_sb, j0, j1):
                # batches 8q+j0 .. 8q+j1 of quarter q
                nb = j1 - j0
                for hh in range(2):  # h' = partition half
                    rows = slice(64 * hh, 64 * hh + 64)
                    in0 = psav[rows,
                               256 * q + 32 * j0:256 * q + 32 * j1].rearrange(
                        "p (b hp hh t) -> p b hp hh t", b=nb, hp=2, t=8)[
                        :, :, :, hh, :]
                    in1 = bc_sb[rows, 32 * j0:32 * j1].rearrange(
                        "p (b hp hh t) -> p b hp hh t", b=nb, hp=2, t=8)[
                        :, :, :, hh, :]
                    out_ap = aoT[rows, :,
                                 64 * q + 8 * j0:64 * q + 8 * j1].rearrange(
                        "p a (b t) -> p b a t", t=8)
                    nc.vector.tensor_tensor(out_ap, in0, in1, ALU.mult)

            def evac_quarter(q):
                evac_range(q, make_bc(q), 0, 8)

            # ---- project one token-half (after its two quarters evac'd) ----
            def out_proj_half(half):
                psos = [ps.tile([128, 512], F32, name=f"pso_{half}_{ob}",
                                tag="ps") for ob in range(4)]
                for a in range(2):  # groups interleave across the 4 banks
                    for ob in range(4):
                        nc.tensor.matmul(
                            psos[ob], aoT[:, a, 128 * half:128 * half + 128],
                            wo_sb[:, a, 512 * ob:512 * ob + 512],
                            start=(a == 0), stop=(a == 1))
                for ob in range(4):
                    osb = small.tile([128, 512], BF16, name=f"osb_{half}_{ob}",
                                     tag="osb", bufs=8)
                    if bo_zero:
                        # bo == 0: plain psum evac, split DVE/ACT so the four
                        # chains drain two-wide in the tail
                        if ob % 2 == 0:
                            nc.vector.tensor_copy(osb, psos[ob])
                        else:
                            nc.scalar.copy(osb, psos[ob])
                    else:
                        nc.vector.tensor_add(osb, psos[ob],
                                             bo_bc[:, 512 * ob:512 * ob + 512])
                    if half == 0:  # Pool carries the vt stream: keep it clear
                        eng = [nc.sync, nc.sync, nc.sync, nc.scalar][ob]
                    else:
                        eng = [nc.sync, nc.gpsimd, nc.scalar, nc.sync][ob]
                    eng.dma_start(
                        out[128 * half:128 * half + 128, 512 * ob:512 * ob + 512],
                        osb)

            # ---- attention (per batch; last 4 scores ran up front) ----
            for b in range(B - len(HOIST)):
                kt = stream.tile([128, 2, 8, 128], BF16, name="kt", tag="kt",
                                 bufs=8)
                vt = stream.tile([128, 8, HC, DH], F8E3, name="vt", tag="vt",
                                 bufs=8)
                nc.gpsimd.dma_start(kt, kT[b])
                nc.sync.dma_start(vt[0:P, :, :, :], v[b])
                # new V rows for this batch land on partition 127:
                # vt[127, i, h, d] = vnew[token 8b+i][64h+d]
                m, r0 = b // 16, 8 * (b % 16)
                nc.gpsimd.dma_start(vt[P:128, :, :, :], vnew[m][r0:r0 + 8, :])
                # new K columns: kt[:, m, i, 127] = knew[:, m, 8b+i]
                nc.vector.tensor_copy(kt[:, :, :, 127], knew[:, :, 8 * b:8 * b + 8])

                # scores^T: scT[p, i, (h,t)] = q . k(8p+i) / 8 (pre-scale in exp)
                sc = ps.tile([128, 512], F32, name="sc", tag="ps")
                scT = sc[:, :256].rearrange("p (i w) -> p i w", w=32)
                for i in range(8):
                    for m2 in range(2):
                        nc.tensor.matmul(
                            scT[:, i, :], kt[:, m2, i, :],
                            qbd[:, m2, 32 * b:32 * b + 32],
                            start=(m2 == 0), stop=(m2 == 1))

                attnT = stream.tile([128, 8, 32], BF16, name="attnT", tag="at",
                                    bufs=3)
                nc.scalar.activation(attnT, scT, AF.Exp, scale=0.125)

                # AV^T: psav[64h'+d, (b, hp, h', t)] += vt^T @ attnT
                for hp in range(2):
                    col = 32 * b + 16 * hp
                    for i in range(8):
                        nc.tensor.matmul(
                            psav[:, col:col + 16],
                            vt[:, i, 2 * hp:2 * hp + 2, :],
                            attnT[:, i, 16 * hp:16 * hp + 16],
                            start=(i == 0), stop=(i == 7))

                # Z[(h,t)] = sum_s exp: ones-matmul over partitions, acc over i
                zt = ps.tile([128, 512], F32, name="zt", tag="ps")
                for i in range(8):
                    nc.tensor.matmul(zt[0:1, 0:32], ones_col, attnT[:, i, :],
                                     start=(i == 0), stop=(i == 7))
                with nc.allow_low_precision(reason="1/Z in f16: 0.05% rel err"):
                    nc.vector.reciprocal(recip_all[:, 32 * b:32 * b + 32],
                                         zt[0:1, 0:32])

                if b % 8 == 7:
                    evac_quarter(b // 8)
                if b == 15:
                    out_proj_half(0)

            # ---- hoisted batches' tails: only AV depends on the vt stream.
            # bc for quarter 3 is ready before the stream drains; the psav
            # columns of b24..30 evacuate behind AV(30), so after the final
            # vt transfer only AV(31) + 32 evac columns + projection remain.
            bc3 = make_bc(3)
            for b in HOIST:
                vt = stream.tile([128, 8, HC, DH], F8E3, name="vt", tag="vt",
                                 bufs=8)
                nc.gpsimd.dma_start(vt[0:P, :, :, :], v[b])
                m, r0 = b // 16, 8 * (b % 16)
                nc.gpsimd.dma_start(vt[P:128, :, :, :], vnew[m][r0:r0 + 8, :])
                for hp in range(2):
                    col = 32 * b + 16 * hp
                    for i in range(8):
                        nc.tensor.matmul(
                            psav[:, col:col + 16],
                            vt[:, i, 2 * hp:2 * hp + 2, :],
                            attnT_h[b][:, i, 16 * hp:16 * hp + 16],
                            start=(i == 0), stop=(i == 7))
                if b == B - 2:
                    evac_range(3, bc3, 0, 7)
            evac_range(3, bc3, 7, 8)
            out_proj_half(1)

    nc.finalize()
    return nc


def _prep_core(c, x_flat_T, cache_keys, cache_values, Wq, bq, Wk, bk, Wv, bv, Wo, bo):
    hs = slice(HC * c, HC * c + HC)
    qs = slice(QD * c, QD * c + QD)

    def perm_rows(W):
        # rows ordered (m, h, j): row 32h + j of tile m = W[64h + 32m + j]
        Ws = W[qs].reshape(HC, 2, 32, -1)              # [h, m, j, d]
        return Ws.transpose(1, 0, 2, 3).reshape(QD, -1)  # [(m,h,j), d]

    wq_p = perm_rows(Wq)
    wk_p = perm_rows(Wk)
    bq_p = np.ascontiguousarray(perm_rows(bq[:, None])[:, 0])
    bk_p = np.ascontiguousarray(perm_rows(bk[:, None])[:, 0])

    def as_tiles(WT):  # [D, 256] -> [128, 16, 256]
        return np.ascontiguousarray(
            WT.reshape(16, 128, QD).transpose(1, 0, 2)).astype(bfloat16)

    wqT = as_tiles(np.ascontiguousarray(wq_p.T))
    wkT = as_tiles(np.ascontiguousarray(wk_p.T))
    wvT = as_tiles(np.ascontiguousarray(Wv[qs].T))
    woT = np.ascontiguousarray(
        Wo[:, qs].T.reshape(2, 128, D).transpose(1, 0, 2)).astype(bfloat16)

    # kT[b, 32h+j, m, i, p] = K[b, h, 8p+i, 32m+j]; p=127 filled on device
    ck = cache_keys[:, hs]                        # [B, 4, 1016, 64]
    km = ck.reshape(B, HC, P, 8, 2, 32)           # [b, h, p, i, m, j]
    kT = np.zeros((B, HC, 32, 2, 8, 128), dtype=bfloat16)  # [b, h, j, m, i, p]
    kT[..., :P] = km.transpose(0, 1, 5, 4, 3, 2).astype(bfloat16)
    kT = kT.reshape(B, 128, 2, 8, 128)

    # v[b, p, i, h, d] = V[b, h, 8p+i, d]
    cv = cache_values[:, hs].reshape(B, HC, P, 8, DH)
    vv = np.ascontiguousarray(cv.transpose(0, 2, 3, 1, 4)).astype(float8_e3m4)

    return {
        "xT": x_flat_T.astype(bfloat16),
        "wqT": wqT, "wkT": wkT, "wvT": wvT, "woT": woT,
        "bq": bq_p.astype(np.float32), "bk": bk_p.astype(np.float32),
        "bv": np.ascontiguousarray(bv[qs]).astype(np.float32),
        "bo": bo.astype(bfloat16),
        "kT": kT,
        "v": vv,
    }


_NC_CACHE = {}


def kernel(x, cache_keys, cache_values, Wq, bq, Wk, bk, Wv, bv, Wo, bo):
    x = np.asarray(x, dtype=np.float32)
    cache_keys = np.asarray(cache_keys, dtype=np.float32)
    cache_values = np.asarray(cache_values, dtype=np.float32)
    Wq, Wk, Wv, Wo = (np.asarray(w, dtype=np.float32) for w in (Wq, Wk, Wv, Wo))
    bq, bk, bv, bo = (np.asarray(b_, dtype=np.float32) for b_ in (bq, bk, bv, bo))

    x_flat_T = np.ascontiguousarray(
        x.reshape(TOK, D).T.reshape(16, 128, TOK).transpose(1, 0, 2))  # [128,16,256]

    in_maps = [
        _prep_core(c, x_flat_T, cache_keys, cache_values,
                   Wq, bq, Wk, bk, Wv, bv, Wo, bo)
        for c in range(N_CORES)
    ]

    cfg = dict(CFG)
    cfg["bo_zero"] = not np.any(bo)
    key = tuple(sorted(cfg.items()))
    if key not in _NC_CACHE:
        _NC_CACHE[key] = build_nc(cfg)
    nc = _NC_CACHE[key]

    res = bass_utils.run_bass_kernel_spmd(nc, in_maps, core_ids=list(range(N_CORES)))
    out = np.zeros((TOK, D), dtype=np.float32)
    for r in res.results:
        out += r["out"].astype(np.float32)
    return out.reshape(B, T, D)


# revision 71
# speedup vs baseline: 1.1892x; 1.0010x over previous
"""Trainium2 Bass kernel for a single-layer MHA decode step with KV cache.

Problem (hardcoded from spec):
  x            [32, 8, 2048]      query tokens (B=32 batches x T=8 steps)
  cache_keys   [32, 32, 1016, 64] (B, H, S_cache, Dh)
  cache_values [32, 32, 1016, 64]
  Wq/Wk/Wv/Wo  [2048, 2048], biases [2048]
  out = MHA(x, cache) @ Wo.T + bo   -> [32, 8, 2048]

Sharding: tensor-parallel over heads. Each of the 8 cores handles 4 heads:
QKV projections for its head slice, attention over its KV-cache slice, and a
partial output projection (rank-256 slice of Wo). Host sums the 8 partials.

Design notes (v3 - transposed attention, bf16 K / fp8-e3m4 V streaming):
 - The kernel is HBM-bound on the KV cache. Keys, weights, x and output
   partials stream as bf16; the VALUE cache streams as fp8 e3m4 (4 mantissa
   bits) and feeds the AV matmul as the stationary operand against bf16
   attention weights (mixed non-fp32 matmul dtypes are allowed). Measured
   rel-err 1.08e-2 against the 2e-2 gate; e4m3 or fp8 keys would exceed it.
   Cost-model time 101.7 us with 88.3 us of DMA transfers (the byte floor
   at the modeled 360 GB/s).
 - Scores are computed TRANSPOSED: scT[s, (h,t)] per batch, with the key
   tile as the stationary matmul operand and the block-diagonal q as the
   moving operand (free dim 32).  s lives on partitions as s = 8p + i with
   i = 0..7 the free-dim chunk; p = 127 holds the 8 freshly projected keys
   (token 8b+i at chunk i), copied into the key tile on device.
 - Softmax: no max subtraction (scores are O(1), exp is safe in f32/bf16).
   exp runs on ACT into bf16 attnT; the normalizer Z per query is a
   ones-vector matmul over partitions; normalization is deferred to the
   psav evacuation (out = (1/Z) * sum exp*v factorizes).
 - AV is also transposed: out[dh, t] with v stationary, attn moving
   (free dim 16 covering a pair of heads), accumulated over the 8 s-chunks
   into one persistent [128, 1024] PSUM region laid out as aoT.
 - Output projection reads aoT directly; partials are written as bf16 and
   summed on host. It runs per token-half: half 0 mid-stream (hidden),
   half 1 in the tail.
 - Issue queues: kt on SP (HWDGE), vt + vnew on Pool (SWDGE, no HWDGE
   contention), exp/evac on ACT. Each DMA issue holds its queue's SEQ
   through desc-gen (~1.2 us), so one queue cannot feed both big streams.
 - Tail: the last 4 batches' key tiles load up front and their score
   chains run early, so after the final vt transfer only AV + a 32-column
   evac + the half-1 projection remain. 14 dummy matmuls at t=0 warm the
   PE p-state so QKV runs at full clock.
"""

import numpy as np
from ml_dtypes import bfloat16, float8_e3m4

import concourse.bass as bass
import concourse.mybir as mybir
import concourse.tile as tile
from concourse import bacc
from concourse import bass_utils

F32 = mybir.dt.float32
F16 = mybir.dt.float16
BF16 = mybir.dt.bfloat16
F8E3 = mybir.dt.float8e3

B, T, D = 32, 8, 2048
H, DH = 32, 64
S_CACHE, S = 1016, 1024
N_CORES = 8
HC = H // N_CORES          # heads per core = 4
TOK = B * T                # 256
QD = HC * DH               # 256 per-core qkv dims
P = 127                    # s-rows per chunk from the cache (1016 = 8*127)

AF = mybir.ActivationFunctionType
ALU = mybir.AluOpType
AX = mybir.AxisListType

CFG = {"dtype": "bf16", "bo_zero": True}


def build_nc(cfg=CFG):
    bo_zero = cfg.get("bo_zero", False)
    nc = bacc.Bacc(None, target_bir_lowering=False)

    xT = nc.dram_tensor("xT", [128, 16, 256], BF16, kind="ExternalInput")
    wqT = nc.dram_tensor("wqT", [128, 16, 256], BF16, kind="ExternalInput")
    wkT = nc.dram_tensor("wkT", [128, 16, 256], BF16, kind="ExternalInput")
    wvT = nc.dram_tensor("wvT", [128, 16, 256], BF16, kind="ExternalInput")
    woT = nc.dram_tensor("woT", [128, 2, 2048], BF16, kind="ExternalInput")
    bq = nc.dram_tensor("bq", [256], F32, kind="ExternalInput")
    bk = nc.dram_tensor("bk", [256], F32, kind="ExternalInput")
    bv = nc.dram_tensor("bv", [256], F32, kind="ExternalInput")
    bo = nc.dram_tensor("bo", [2048], BF16, kind="ExternalInput")
    # kT[b, (h,j), m, i, p]: keys with s = 8p+i on tile axes; p=127 is filled
    # on device with the new key of token 8b+i.
    kT = nc.dram_tensor("kT", [B, 128, 2, 8, 128], BF16, kind="ExternalInput")
    # v[b, p, i, h, dh] = cache_values[b, h, 8p+i, dh]
    v = nc.dram_tensor("v", [B, P, 8, HC, DH], F8E3, kind="ExternalInput")
    out = nc.dram_tensor("out", [TOK, D], BF16, kind="ExternalOutput")

    with tile.TileContext(nc) as tc:
        with (
            tc.tile_pool(name="singles", bufs=1) as singles,
            tc.tile_pool(name="stream", bufs=8) as stream,
            tc.tile_pool(name="small", bufs=8) as small,
            tc.tile_pool(name="ps", bufs=6, space="PSUM") as ps,
            tc.tile_pool(name="ps_av", bufs=1, space="PSUM") as ps_av,
        ):
            # ---- persistent tiles ----
            xT_sb = singles.tile([128, 16, 256], BF16)
            wq_sb = singles.tile([128, 16, 256], BF16)
            wk_sb = singles.tile([128, 16, 256], BF16)
            wv_sb = singles.tile([128, 16, 256], BF16)
            wo_sb = singles.tile([128, 2, 2048], BF16)
            # weight loads split across issue queues: SP and Act alternate so
            # the shared HWDGE stage doesn't serialize one queue's prefetch
            nc.sync.dma_start(xT_sb, xT[:, :, :])
            nc.scalar.dma_start(wq_sb, wqT[:, :, :])
            nc.sync.dma_start(wk_sb, wkT[:, :, :])
            nc.scalar.dma_start(wv_sb, wvT[:, :, :])
            bq_sb = singles.tile([128, 2], F32)
            bk_sb = singles.tile([128, 2], F32)
            nc.gpsimd.dma_start(bq_sb, bq[:].rearrange("(m p) -> p m", p=128))
            nc.gpsimd.dma_start(bk_sb, bk[:].rearrange("(m p) -> p m", p=128))
            bv_bc = singles.tile([128, 256], F32)
            nc.gpsimd.dma_start(
                bv_bc, bass.AP(tensor=bv[:].tensor, offset=0, ap=[[0, 128], [1, 256]])
            )
            nc.gpsimd.dma_start(wo_sb, woT[:, :, :])
            # the LAST batches' keys load up front: their scores/exp/Z/recip
            # run early, so the tail after the final vt transfers is just
            # AV + evac + projection (no softmax chain on the critical path)
            HOIST = [28, 29, 30, 31]
            kt_h = {}
            for b in HOIST:
                kt_h[b] = singles.tile([128, 2, 8, 128], BF16, name=f"kt_h{b}")
                nc.sync.dma_start(kt_h[b], kT[b])
            if not bo_zero:
                bo_bc = singles.tile([128, 2048], BF16)
                nc.gpsimd.dma_start(
                    bo_bc,
                    bass.AP(tensor=bo[:].tensor, offset=0, ap=[[0, 128], [1, 2048]])
                )

            # Q in block-diag layout: qbd[32h+j, m, (b, 8h'+t)]
            qbd = singles.tile([128, 2, 1024], BF16)
            nc.vector.memset(qbd, 0.0)
            knew = singles.tile([128, 2, 256], BF16)  # [(h,j), m, tok]
            ones_col = singles.tile([128, 1], BF16)
            nc.vector.memset(ones_col, 1.0)
            ones_row = singles.tile([1, 128], F16)
            nc.vector.memset(ones_row, 1.0)
            recip_all = singles.tile([1, 1024], F16)  # 1/Z per (b, h, t)
            aoT = singles.tile([128, 2, 256], BF16)   # [64h'+dh, hp, tok]

            # persistent AV accumulator: [64h'+dh, (b, hp, h', t)]
            psav = ps_av.tile([128, 1024], F32)

            # ---- PE p-state warmup: keep the tensor engine continuously busy
            # while weights stream in, so QKV matmuls run at full clock.
            # Results land in psav rows 0-1, later cleared by AV's start=True.
            warm = singles.tile([128, 512], BF16)
            nc.vector.memset(warm, 0.0)
            for w in range(14):
                nc.tensor.matmul(psav[0:1, 0:512], ones_col, warm,
                                 start=True, stop=True)

            # ---- projections ----
            for m in range(2):
                psq = ps.tile([128, 512], F32, name=f"psq_{m}", tag="ps")[:, :256]
                psk = ps.tile([128, 512], F32, name=f"psk_{m}", tag="ps")[:, :256]
                for k in range(16):
                    st = dict(start=(k == 0), stop=(k == 15))
                    nc.tensor.matmul(
                        psq, wq_sb[:, k, 128 * m:128 * m + 128],
                        xT_sb[:, k, :], **st)
                for k in range(16):
                    st = dict(start=(k == 0), stop=(k == 15))
                    nc.tensor.matmul(
                        psk, wk_sb[:, k, 128 * m:128 * m + 128],
                        xT_sb[:, k, :], **st)
                # evac Q into block-diag (strided) + bias; psum rows 32h+j
                for h in range(4):
                    rows = slice(32 * h, 32 * h + 32)
                    out_ap = qbd[rows, m, :].rearrange("p (b w) -> p b w", w=32)[
                        :, :, 8 * h:8 * h + 8
                    ]
                    in_ap = psq[rows, :].rearrange("p (b t) -> p b t", t=8)
                    nc.scalar.activation(out_ap, in_ap, AF.Identity,
                                         bias=bq_sb[rows, m:m + 1], scale=1.0)
                nc.scalar.activation(knew[:, m, :], psk, AF.Identity,
                                     bias=bk_sb[:, m:m + 1], scale=1.0)

            vnew = []
            for m in range(2):
                psv = ps.tile([128, 512], F32, name=f"psv_{m}", tag="ps")[:, :256]
                for k in range(16):
                    st = dict(start=(k == 0), stop=(k == 15))
                    nc.tensor.matmul(
                        psv, xT_sb[:, k, 128 * m:128 * m + 128],
                        wv_sb[:, k, :], **st)
                vnew_sb = small.tile([128, 256], F8E3, name=f"vnew_sb_{m}",
                                     tag="vnew", bufs=2)
                with nc.allow_low_precision(reason="V stream is fp8 e3m4"):
                    nc.vector.tensor_add(vnew_sb, psv, bv_bc)
                vnew.append(vnew_sb)

            # ---- hoisted batches' score chains, run up front ----
            attnT_h = {}
            for b in HOIST:
                attnT_h[b] = singles.tile([128, 8, 32], BF16, name=f"at_h{b}")
                nc.vector.tensor_copy(kt_h[b][:, :, :, 127],
                                      knew[:, :, 8 * b:8 * b + 8])
                sch = ps.tile([128, 512], F32, name=f"sc_h{b}", tag="ps")
                scTh = sch[:, :256].rearrange("p (i w) -> p i w", w=32)
                for i in range(8):
                    for m2 in range(2):
                        nc.tensor.matmul(
                            scTh[:, i, :], kt_h[b][:, m2, i, :],
                            qbd[:, m2, 32 * b:32 * b + 32],
                            start=(m2 == 0), stop=(m2 == 1))
                nc.scalar.activation(attnT_h[b], scTh, AF.Exp, scale=0.125)
                zth = ps.tile([128, 512], F32, name=f"zt_h{b}", tag="ps")
                for i in range(8):
                    nc.tensor.matmul(zth[0:1, 0:32], ones_col,
                                     attnT_h[b][:, i, :],
                                     start=(i == 0), stop=(i == 7))
                with nc.allow_low_precision(reason="1/Z in f16"):
                    nc.vector.reciprocal(recip_all[:, 32 * b:32 * b + 32],
                                         zth[0:1, 0:32])

            # ---- normalize + evacuate a quarter of psav into aoT ----
            def make_bc(q):
                bc = ps.tile([128, 512], F32, name=f"bc_{q}", tag="ps")[:, :256]
                nc.tensor.matmul(bc, ones_row,
                                 recip_all[:, 256 * q:256 * q + 256],
                                 start=True, stop=True)
                bc_sb = small.tile([128, 256], F16, name=f"bc_sb_{q}",
                                   tag="bcs", bufs=2)
                nc.scalar.copy(bc_sb, bc)
                return bc_sb

            def evac_range(q, bc_sb, j0, j1):
                # batches 8q+j0 .. 8q+j1 of quarter q
                nb = j1 - j0
                for hh in range(2):  # h' = partition half
                    rows = slice(64 * hh, 64 * hh + 64)
                    in0 = psav[rows,
                               256 * q + 32 * j0:256 * q + 32 * j1].rearrange(
                        "p (b hp hh t) -> p b hp hh t", b=nb, hp=2, t=8)[
                        :, :, :, hh, :]
                    in1 = bc_sb[rows, 32 * j0:32 * j1].rearrange(
                        "p (b hp hh t) -> p b hp hh t", b=nb, hp=2, t=8)[
                        :, :, :, hh, :]
                    out_ap = aoT[rows, :,
                                 64 * q + 8 * j0:64 * q + 8 * j1].rearrange(
                        "p a (b t) -> p b a t", t=8)
                    nc.vector.tensor_tensor(out_ap, in0, in1, ALU.mult)

            def evac_quarter(q):
                evac_range(q, make_bc(q), 0, 8)

            # ---- project one token-half (after its two quarters evac'd) ----
            def out_proj_half(half):
                psos = [ps.tile([128, 512], F32, name=f"pso_{half}_{ob}",
                                tag="ps") for ob in range(4)]
                for a in range(2):  # groups interleave across the 4 banks
                    for ob in range(4):
                        nc.tensor.matmul(
                            psos[ob], aoT[:, a, 128 * half:128 * half + 128],
                            wo_sb[:, a, 512 * ob:512 * ob + 512],
                            start=(a == 0), stop=(a == 1))
                for ob in range(4):
                    osb = small.tile([128, 512], BF16, name=f"osb_{half}_{ob}",
                                     tag="osb", bufs=8)
                    if bo_zero:
                        # bo == 0: plain psum evac, split DVE/ACT so the four
                        # chains drain two-wide in the tail
                        if ob % 2 == 0:
                            nc.vector.tensor_copy(osb, psos[ob])
                        else:
                            nc.scalar.copy(osb, psos[ob])
                    else:
                        nc.vector.tensor_add(osb, psos[ob],
                                             bo_bc[:, 512 * ob:512 * ob + 512])
                    if half == 0:  # Pool carries the vt stream: keep it clear
                        eng = [nc.sync, nc.sync, nc.sync, nc.scalar][ob]
                    else:
                        eng = [nc.sync, nc.gpsimd, nc.scalar, nc.sync][ob]
                    eng.dma_start(
                        out[128 * half:128 * half + 128, 512 * ob:512 * ob + 512],
                        osb)

            # ---- attention (per batch; last 4 scores ran up front) ----
            for b in range(B - len(HOIST)):
                kt = stream.tile([128, 2, 8, 128], BF16, name="kt", tag="kt",
                                 bufs=8)
                vt = stream.tile([128, 8, HC, DH], F8E3, name="vt", tag="vt",
                                 bufs=8)
                nc.gpsimd.dma_start(kt, kT[b])
                nc.sync.dma_start(vt[0:P, :, :, :], v[b])
                # new V rows for this batch land on partition 127:
                # vt[127, i, h, d] = vnew[token 8b+i][64h+d]
                m, r0 = b // 16, 8 * (b % 16)
                nc.gpsimd.dma_start(vt[P:128, :, :, :], vnew[m][r0:r0 + 8, :])
                # new K columns: kt[:, m, i, 127] = knew[:, m, 8b+i]
                nc.vector.tensor_copy(kt[:, :, :, 127], knew[:, :, 8 * b:8 * b + 8])

                # scores^T: scT[p, i, (h,t)] = q . k(8p+i) / 8 (pre-scale in exp)
                sc = ps.tile([128, 512], F32, name="sc", tag="ps")
                scT = sc[:, :256].rearrange("p (i w) -> p i w", w=32)
                for i in range(8):
                    for m2 in range(2):
                        nc.tensor.matmul(
                            scT[:, i, :], kt[:, m2, i, :],
                            qbd[:, m2, 32 * b:32 * b + 32],
                            start=(m2 == 0), stop=(m2 == 1))

                attnT = stream.tile([128, 8, 32], BF16, name="attnT", tag="at",
                                    bufs=3)
                nc.scalar.activation(attnT, scT, AF.Exp, scale=0.125)

                # AV^T: psav[64h'+d, (b, hp, h', t)] += vt^T @ attnT
                for hp in range(2):
                    col = 32 * b + 16 * hp
                    for i in range(8):
                        nc.tensor.matmul(
                            psav[:, col:col + 16],
                            vt[:, i, 2 * hp:2 * hp + 2, :],
                            attnT[:, i, 16 * hp:16 * hp + 16],
                            start=(i == 0), stop=(i == 7))

                # Z[(h,t)] = sum_s exp: ones-matmul over partitions, acc over i
                zt = ps.tile([128, 512], F32, name="zt", tag="ps")
                for i in range(8):
                    nc.tensor.matmul(zt[0:1, 0:32], ones_col, attnT[:, i, :],
                                     start=(i == 0), stop=(i == 7))
                with nc.allow_low_precision(reason="1/Z in f16: 0.05% rel err"):
                    nc.vector.reciprocal(recip_all[:, 32 * b:32 * b + 32],
                                         zt[0:1, 0:32])

                if b % 8 == 7:
                    evac_quarter(b // 8)
                if b == 15:
                    out_proj_half(0)

            # ---- hoisted batches' tails: only AV depends on the vt stream.
            # bc for quarter 3 is ready before the stream drains; the psav
            # columns of b24..30 evacuate behind AV(30), so after the final
            # vt transfer only AV(31) + 32 evac columns + projection remain.
            bc3 = make_bc(3)
            for b in HOIST:
                vt = stream.tile([128, 8, HC, DH], F8E3, name="vt", tag="vt",
                                 bufs=8)
                (nc.gpsimd if b % 2 == 0 else nc.sync).dma_start(
                    vt[0:P, :, :, :], v[b])
                m, r0 = b // 16, 8 * (b % 16)
                nc.gpsimd.dma_start(vt[P:128, :, :, :], vnew[m][r0:r0 + 8, :])
                for hp in range(2):
                    col = 32 * b + 16 * hp
                    for i in range(8):
                        nc.tensor.matmul(
                            psav[:, col:col + 16],
                            vt[:, i, 2 * hp:2 * hp + 2, :],
                            attnT_h[b][:, i, 16 * hp:16 * hp + 16],
                            start=(i == 0), stop=(i == 7))
                if b == B - 2:
                    evac_range(3, bc3, 0, 7)
            evac_range(3, bc3, 7, 8)
            out_proj_half(1)

    nc.finalize()
    return nc


def _prep_core(c, x_flat_T, cache_keys, cache_values, Wq, bq, Wk, bk, Wv, bv, Wo, bo):
    hs = slice(HC * c, HC * c + HC)
    qs = slice(QD * c, QD * c + QD)

    def perm_rows(W):
        # rows ordered (m, h, j): row 32h + j of tile m = W[64h + 32m + j]
        Ws = W[qs].reshape(HC, 2, 32, -1)              # [h, m, j, d]
        return Ws.transpose(1, 0, 2, 3).reshape(QD, -1)  # [(m,h,j), d]

    wq_p = perm_rows(Wq)
    wk_p = perm_rows(Wk)
    bq_p = np.ascontiguousarray(perm_rows(bq[:, None])[:, 0])
    bk_p = np.ascontiguousarray(perm_rows(bk[:, None])[:, 0])

    def as_tiles(WT):  # [D, 256] -> [128, 16, 256]
        return np.ascontiguousarray(
            WT.reshape(16, 128, QD).transpose(1, 0, 2)).astype(bfloat16)

    wqT = as_tiles(np.ascontiguousarray(wq_p.T))
    wkT = as_tiles(np.ascontiguousarray(wk_p.T))
    wvT = as_tiles(np.ascontiguousarray(Wv[qs].T))
    woT = np.ascontiguousarray(
        Wo[:, qs].T.reshape(2, 128, D).transpose(1, 0, 2)).astype(bfloat16)

    # kT[b, 32h+j, m, i, p] = K[b, h, 8p+i, 32m+j]; p=127 filled on device
    ck = cache_keys[:, hs]                        # [B, 4, 1016, 64]
    km = ck.reshape(B, HC, P, 8, 2, 32)           # [b, h, p, i, m, j]
    kT = np.zeros((B, HC, 32, 2, 8, 128), dtype=bfloat16)  # [b, h, j, m, i, p]
    kT[..., :P] = km.transpose(0, 1, 5, 4, 3, 2).astype(bfloat16)
    kT = kT.reshape(B, 128, 2, 8, 128)

    # v[b, p, i, h, d] = V[b, h, 8p+i, d]
    cv = cache_values[:, hs].reshape(B, HC, P, 8, DH)
    vv = np.ascontiguousarray(cv.transpose(0, 2, 3, 1, 4)).astype(float8_e3m4)

    return {
        "xT": x_flat_T.astype(bfloat16),
        "wqT": wqT, "wkT": wkT, "wvT": wvT, "woT": woT,
        "bq": bq_p.astype(np.float32), "bk": bk_p.astype(np.float32),
        "bv": np.ascontiguousarray(bv[qs]).astype(np.float32),
        "bo": bo.astype(bfloat16),
        "kT": kT,
        "v": vv,
    }


_NC_CACHE = {}


def kernel(x, cache_keys, cache_values, Wq, bq, Wk, bk, Wv, bv, Wo, bo):
    x = np.asarray(x, dtype=np.float32)
    cache_keys = np.asarray(cache_keys, dtype=np.float32)
    cache_values = np.asarray(cache_values, dtype=np.float32)
    Wq, Wk, Wv, Wo = (np.asarray(w, dtype=np.float32) for w in (Wq, Wk, Wv, Wo))
    bq, bk, bv, bo = (np.asarray(b_, dtype=np.float32) for b_ in (bq, bk, bv, bo))

    x_flat_T = np.ascontiguousarray(
        x.reshape(TOK, D).T.reshape(16, 128, TOK).transpose(1, 0, 2))  # [128,16,256]

    in_maps = [
        _prep_core(c, x_flat_T, cache_keys, cache_values,
                   Wq, bq, Wk, bk, Wv, bv, Wo, bo)
        for c in range(N_CORES)
    ]

    cfg = dict(CFG)
    cfg["bo_zero"] = not np.any(bo)
    key = tuple(sorted(cfg.items()))
    if key not in _NC_CACHE:
        _NC_CACHE[key] = build_nc(cfg)
    nc = _NC_CACHE[key]

    res = bass_utils.run_bass_kernel_spmd(nc, in_maps, core_ids=list(range(N_CORES)))
    out = np.zeros((TOK, D), dtype=np.float32)
    for r in res.results:
        out += r["out"].astype(np.float32)
    return out.reshape(B, T, D)


# revision 74
# speedup vs baseline: 1.1900x; 1.0007x over previous
"""Trainium2 Bass kernel for a single-layer MHA decode step with KV cache.

Problem (hardcoded from spec):
  x            [32, 8, 2048]      query tokens (B=32 batches x T=8 steps)
  cache_keys   [32, 32, 1016, 64] (B, H, S_cache, Dh)
  cache_values [32, 32, 1016, 64]
  Wq/Wk/Wv/Wo  [2048, 2048], biases [2048]
  out = MHA(x, cache) @ Wo.T + bo   -> [32, 8, 2048]

Sharding: tensor-parallel over heads. Each of the 8 cores handles 4 heads:
QKV projections for its head slice, attention over its KV-cache slice, and a
partial output projection (rank-256 slice of Wo). Host sums the 8 partials.

Design notes (v3 - transposed attention, bf16 K / fp8-e3m4 V streaming):
 - The kernel is HBM-bound on the KV cache. Keys, weights, x and output
   partials stream as bf16; the VALUE cache streams as fp8 e3m4 (4 mantissa
   bits) and feeds the AV matmul as the stationary operand against bf16
   attention weights (mixed non-fp32 matmul dtypes are allowed). Measured
   rel-err 1.08e-2 against the 2e-2 gate; e4m3 or fp8 keys would exceed it.
   Cost-model time 101.7 us with 88.3 us of DMA transfers (the byte floor
   at the modeled 360 GB/s).
 - Scores are computed TRANSPOSED: scT[s, (h,t)] per batch, with the key
   tile as the stationary matmul operand and the block-diagonal q as the
   moving operand (free dim 32).  s lives on partitions as s = 8p + i with
   i = 0..7 the free-dim chunk; p = 127 holds the 8 freshly projected keys
   (token 8b+i at chunk i), copied into the key tile on device.
 - Softmax: no max subtraction (scores are O(1), exp is safe in f32/bf16).
   exp runs on ACT into bf16 attnT; the normalizer Z per query is a
   ones-vector matmul over partitions; normalization is deferred to the
   psav evacuation (out = (1/Z) * sum exp*v factorizes).
 - AV is also transposed: out[dh, t] with v stationary, attn moving
   (free dim 16 covering a pair of heads), accumulated over the 8 s-chunks
   into one persistent [128, 1024] PSUM region laid out as aoT.
 - Output projection reads aoT directly; partials are written as bf16 and
   summed on host. It runs per token-half: half 0 mid-stream (hidden),
   half 1 in the tail.
 - Issue queues: kt on SP (HWDGE), vt + vnew on Pool (SWDGE, no HWDGE
   contention), exp/evac on ACT. Each DMA issue holds its queue's SEQ
   through desc-gen (~1.2 us), so one queue cannot feed both big streams.
 - Tail: the last 4 batches' key tiles load up front and their score
   chains run early, so after the final vt transfer only AV + a 32-column
   evac + the half-1 projection remain. 14 dummy matmuls at t=0 warm the
   PE p-state so QKV runs at full clock.
"""

import numpy as np
from ml_dtypes import bfloat16, float8_e3m4

import concourse.bass as bass
import concourse.mybir as mybir
import concourse.tile as tile
from concourse import bacc
from concourse import bass_utils

F32 = mybir.dt.float32
F16 = mybir.dt.float16
BF16 = mybir.dt.bfloat16
F8E3 = mybir.dt.float8e3

B, T, D = 32, 8, 2048
H, DH = 32, 64
S_CACHE, S = 1016, 1024
N_CORES = 8
HC = H // N_CORES          # heads per core = 4
TOK = B * T                # 256
QD = HC * DH               # 256 per-core qkv dims
P = 127                    # s-rows per chunk from the cache (1016 = 8*127)

AF = mybir.ActivationFunctionType
ALU = mybir.AluOpType
AX = mybir.AxisListType

CFG = {"dtype": "bf16", "bo_zero": True}


def build_nc(cfg=CFG):
    bo_zero = cfg.get("bo_zero", False)
    nc = bacc.Bacc(None, target_bir_lowering=False)

    xT = nc.dram_tensor("xT", [128, 16, 256], BF16, kind="ExternalInput")
    wqT = nc.dram_tensor("wqT", [128, 16, 256], BF16, kind="ExternalInput")
    wkT = nc.dram_tensor("wkT", [128, 16, 256], BF16, kind="ExternalInput")
    wvT = nc.dram_tensor("wvT", [128, 16, 256], BF16, kind="ExternalInput")
    woT = nc.dram_tensor("woT", [128, 2, 2048], BF16, kind="ExternalInput")
    bq = nc.dram_tensor("bq", [256], F32, kind="ExternalInput")
    bk = nc.dram_tensor("bk", [256], F32, kind="ExternalInput")
    bv = nc.dram_tensor("bv", [256], F32, kind="ExternalInput")
    bo = nc.dram_tensor("bo", [2048], BF16, kind="ExternalInput")
    # kT[b, (h,j), m, i, p]: keys with s = 8p+i on tile axes; p=127 is filled
    # on device with the new key of token 8b+i.
    kT = nc.dram_tensor("kT", [B, 128, 2, 8, 128], BF16, kind="ExternalInput")
    # v[b, p, i, h, dh] = cache_values[b, h, 8p+i, dh]
    v = nc.dram_tensor("v", [B, P, 8, HC, DH], F8E3, kind="ExternalInput")
    out = nc.dram_tensor("out", [TOK, D], BF16, kind="ExternalOutput")

    with tile.TileContext(nc) as tc:
        with (
            tc.tile_pool(name="singles", bufs=1) as singles,
            tc.tile_pool(name="stream", bufs=8) as stream,
            tc.tile_pool(name="small", bufs=8) as small,
            tc.tile_pool(name="ps", bufs=6, space="PSUM") as ps,
            tc.tile_pool(name="ps_av", bufs=1, space="PSUM") as ps_av,
        ):
            # ---- persistent tiles ----
            xT_sb = singles.tile([128, 16, 256], BF16)
            wq_sb = singles.tile([128, 16, 256], BF16)
            wk_sb = singles.tile([128, 16, 256], BF16)
            wv_sb = singles.tile([128, 16, 256], BF16)
            wo_sb = singles.tile([128, 2, 2048], BF16)
            # weight loads split across issue queues: SP and Act alternate so
            # the shared HWDGE stage doesn't serialize one queue's prefetch
            nc.sync.dma_start(xT_sb, xT[:, :, :])
            nc.scalar.dma_start(wq_sb, wqT[:, :, :])
            nc.sync.dma_start(wk_sb, wkT[:, :, :])
            nc.scalar.dma_start(wv_sb, wvT[:, :, :])
            bq_sb = singles.tile([128, 2], F32)
            bk_sb = singles.tile([128, 2], F32)
            nc.gpsimd.dma_start(bq_sb, bq[:].rearrange("(m p) -> p m", p=128))
            nc.gpsimd.dma_start(bk_sb, bk[:].rearrange("(m p) -> p m", p=128))
            bv_bc = singles.tile([128, 256], F32)
            nc.gpsimd.dma_start(
                bv_bc, bass.AP(tensor=bv[:].tensor, offset=0, ap=[[0, 128], [1, 256]])
            )
            # the LAST batches' keys load up front: their scores/exp/Z/recip
            # run early, so the tail after the final vt transfers is just
            # AV + evac + projection (no softmax chain on the critical path)
            HOIST = [28, 29, 30, 31]
            kt_h = {}
            for b in HOIST:
                kt_h[b] = singles.tile([128, 2, 8, 128], BF16, name=f"kt_h{b}")
                nc.sync.dma_start(kt_h[b], kT[b])
            if not bo_zero:
                bo_bc = singles.tile([128, 2048], BF16)
                nc.gpsimd.dma_start(
                    bo_bc,
                    bass.AP(tensor=bo[:].tensor, offset=0, ap=[[0, 128], [1, 2048]])
                )

            # Q in block-diag layout: qbd[32h+j, m, (b, 8h'+t)]
            qbd = singles.tile([128, 2, 1024], BF16)
            nc.vector.memset(qbd, 0.0)
            knew = singles.tile([128, 2, 256], BF16)  # [(h,j), m, tok]
            ones_col = singles.tile([128, 1], BF16)
            nc.vector.memset(ones_col, 1.0)
            ones_row = singles.tile([1, 128], F16)
            nc.vector.memset(ones_row, 1.0)
            recip_all = singles.tile([1, 1024], F16)  # 1/Z per (b, h, t)
            aoT = singles.tile([128, 2, 256], BF16)   # [64h'+dh, hp, tok]

            # persistent AV accumulator: [64h'+dh, (b, hp, h', t)]
            psav = ps_av.tile([128, 1024], F32)

            # ---- PE p-state warmup: keep the tensor engine continuously busy
            # while weights stream in, so QKV matmuls run at full clock.
            # Results land in psav rows 0-1, later cleared by AV's start=True.
            warm = singles.tile([128, 512], BF16)
            nc.vector.memset(warm, 0.0)
            for w in range(14):
                nc.tensor.matmul(psav[0:1, 0:512], ones_col, warm,
                                 start=True, stop=True)

            # ---- projections ----
            for m in range(2):
                psq = ps.tile([128, 512], F32, name=f"psq_{m}", tag="ps")[:, :256]
                psk = ps.tile([128, 512], F32, name=f"psk_{m}", tag="ps")[:, :256]
                for k in range(16):
                    st = dict(start=(k == 0), stop=(k == 15))
                    nc.tensor.matmul(
                        psq, wq_sb[:, k, 128 * m:128 * m + 128],
                        xT_sb[:, k, :], **st)
                for k in range(16):
                    st = dict(start=(k == 0), stop=(k == 15))
                    nc.tensor.matmul(
                        psk, wk_sb[:, k, 128 * m:128 * m + 128],
                        xT_sb[:, k, :], **st)
                # evac Q into block-diag (strided) + bias; psum rows 32h+j
                for h in range(4):
                    rows = slice(32 * h, 32 * h + 32)
                    out_ap = qbd[rows, m, :].rearrange("p (b w) -> p b w", w=32)[
                        :, :, 8 * h:8 * h + 8
                    ]
                    in_ap = psq[rows, :].rearrange("p (b t) -> p b t", t=8)
                    nc.scalar.activation(out_ap, in_ap, AF.Identity,
                                         bias=bq_sb[rows, m:m + 1], scale=1.0)
                nc.scalar.activation(knew[:, m, :], psk, AF.Identity,
                                     bias=bk_sb[:, m:m + 1], scale=1.0)

            vnew = []
            for m in range(2):
                psv = ps.tile([128, 512], F32, name=f"psv_{m}", tag="ps")[:, :256]
                for k in range(16):
                    st = dict(start=(k == 0), stop=(k == 15))
                    nc.tensor.matmul(
                        psv, xT_sb[:, k, 128 * m:128 * m + 128],
                        wv_sb[:, k, :], **st)
                vnew_sb = small.tile([128, 256], F8E3, name=f"vnew_sb_{m}",
                                     tag="vnew", bufs=2)
                with nc.allow_low_precision(reason="V stream is fp8 e3m4"):
                    nc.vector.tensor_add(vnew_sb, psv, bv_bc)
                vnew.append(vnew_sb)

            # ---- hoisted batches' score chains, run up front ----
            attnT_h = {}
            for b in HOIST:
                attnT_h[b] = singles.tile([128, 8, 32], BF16, name=f"at_h{b}")
                nc.vector.tensor_copy(kt_h[b][:, :, :, 127],
                                      knew[:, :, 8 * b:8 * b + 8])
                sch = ps.tile([128, 512], F32, name=f"sc_h{b}", tag="ps")
                scTh = sch[:, :256].rearrange("p (i w) -> p i w", w=32)
                for i in range(8):
                    for m2 in range(2):
                        nc.tensor.matmul(
                            scTh[:, i, :], kt_h[b][:, m2, i, :],
                            qbd[:, m2, 32 * b:32 * b + 32],
                            start=(m2 == 0), stop=(m2 == 1))
                nc.scalar.activation(attnT_h[b], scTh, AF.Exp, scale=0.125)
                zth = ps.tile([128, 512], F32, name=f"zt_h{b}", tag="ps")
                for i in range(8):
                    nc.tensor.matmul(zth[0:1, 0:32], ones_col,
                                     attnT_h[b][:, i, :],
                                     start=(i == 0), stop=(i == 7))
                with nc.allow_low_precision(reason="1/Z in f16"):
                    nc.vector.reciprocal(recip_all[:, 32 * b:32 * b + 32],
                                         zth[0:1, 0:32])

            # ---- normalize + evacuate a quarter of psav into aoT ----
            def make_bc(q):
                bc = ps.tile([128, 512], F32, name=f"bc_{q}", tag="ps")[:, :256]
                nc.tensor.matmul(bc, ones_row,
                                 recip_all[:, 256 * q:256 * q + 256],
                                 start=True, stop=True)
                bc_sb = small.tile([128, 256], F16, name=f"bc_sb_{q}",
                                   tag="bcs", bufs=2)
                nc.scalar.copy(bc_sb, bc)
                return bc_sb

            def evac_range(q, bc_sb, j0, j1):
                # batches 8q+j0 .. 8q+j1 of quarter q
                nb = j1 - j0
                for hh in range(2):  # h' = partition half
                    rows = slice(64 * hh, 64 * hh + 64)
                    in0 = psav[rows,
                               256 * q + 32 * j0:256 * q + 32 * j1].rearrange(
                        "p (b hp hh t) -> p b hp hh t", b=nb, hp=2, t=8)[
                        :, :, :, hh, :]
                    in1 = bc_sb[rows, 32 * j0:32 * j1].rearrange(
                        "p (b hp hh t) -> p b hp hh t", b=nb, hp=2, t=8)[
                        :, :, :, hh, :]
                    out_ap = aoT[rows, :,
                                 64 * q + 8 * j0:64 * q + 8 * j1].rearrange(
                        "p a (b t) -> p b a t", t=8)
                    nc.vector.tensor_tensor(out_ap, in0, in1, ALU.mult)

            def evac_quarter(q):
                evac_range(q, make_bc(q), 0, 8)

            # ---- project one token-half (after its two quarters evac'd) ----
            def out_proj_half(half):
                psos = [ps.tile([128, 512], F32, name=f"pso_{half}_{ob}",
                                tag="ps") for ob in range(4)]
                for a in range(2):  # groups interleave across the 4 banks
                    for ob in range(4):
                        nc.tensor.matmul(
                            psos[ob], aoT[:, a, 128 * half:128 * half + 128],
                            wo_sb[:, a, 512 * ob:512 * ob + 512],
                            start=(a == 0), stop=(a == 1))
                for ob in range(4):
                    osb = small.tile([128, 512], BF16, name=f"osb_{half}_{ob}",
                                     tag="osb", bufs=8)
                    if bo_zero:
                        # bo == 0: plain psum evac, split DVE/ACT so the four
                        # chains drain two-wide in the tail
                        if ob % 2 == 0:
                            nc.vector.tensor_copy(osb, psos[ob])
                        else:
                            nc.scalar.copy(osb, psos[ob])
                    else:
                        nc.vector.tensor_add(osb, psos[ob],
                                             bo_bc[:, 512 * ob:512 * ob + 512])
                    if half == 0:  # Pool carries the vt stream: keep it clear
                        eng = [nc.sync, nc.sync, nc.sync, nc.scalar][ob]
                    else:
                        eng = [nc.sync, nc.gpsimd, nc.scalar, nc.sync][ob]
                    eng.dma_start(
                        out[128 * half:128 * half + 128, 512 * ob:512 * ob + 512],
                        osb)

            # ---- attention (per batch; last 4 scores ran up front) ----
            for b in range(B - len(HOIST)):
                kt = stream.tile([128, 2, 8, 128], BF16, name="kt", tag="kt",
                                 bufs=8)
                vt = stream.tile([128, 8, HC, DH], F8E3, name="vt", tag="vt",
                                 bufs=8)
                nc.gpsimd.dma_start(kt, kT[b])
                nc.sync.dma_start(vt[0:P, :, :, :], v[b])
                # new V rows for this batch land on partition 127:
                # vt[127, i, h, d] = vnew[token 8b+i][64h+d]
                m, r0 = b // 16, 8 * (b % 16)
                nc.gpsimd.dma_start(vt[P:128, :, :, :], vnew[m][r0:r0 + 8, :])
                if b == 4:  # wo needed only at the b==15 projection
                    nc.gpsimd.dma_start(wo_sb, woT[:, :, :])
                # new K columns: kt[:, m, i, 127] = knew[:, m, 8b+i]
                nc.vector.tensor_copy(kt[:, :, :, 127], knew[:, :, 8 * b:8 * b + 8])

                # scores^T: scT[p, i, (h,t)] = q . k(8p+i) / 8 (pre-scale in exp)
                sc = ps.tile([128, 512], F32, name="sc", tag="ps")
                scT = sc[:, :256].rearrange("p (i w) -> p i w", w=32)
                for i in range(8):
                    for m2 in range(2):
                        nc.tensor.matmul(
                            scT[:, i, :], kt[:, m2, i, :],
                            qbd[:, m2, 32 * b:32 * b + 32],
                            start=(m2 == 0), stop=(m2 == 1))

                attnT = stream.tile([128, 8, 32], BF16, name="attnT", tag="at",
                                    bufs=3)
                nc.scalar.activation(attnT, scT, AF.Exp, scale=0.125)

                # AV^T: psav[64h'+d, (b, hp, h', t)] += vt^T @ attnT
                for hp in range(2):
                    col = 32 * b + 16 * hp
                    for i in range(8):
                        nc.tensor.matmul(
                            psav[:, col:col + 16],
                            vt[:, i, 2 * hp:2 * hp + 2, :],
                            attnT[:, i, 16 * hp:16 * hp + 16],
                            start=(i == 0), stop=(i == 7))

                # Z[(h,t)] = sum_s exp: ones-matmul over partitions, acc over i
                zt = ps.tile([128, 512], F32, name="zt", tag="ps")
                for i in range(8):
                    nc.tensor.matmul(zt[0:1, 0:32], ones_col, attnT[:, i, :],
                                     start=(i == 0), stop=(i == 7))
                with nc.allow_low_precision(reason="1/Z in f16: 0.05% rel err"):
                    nc.vector.reciprocal(recip_all[:, 32 * b:32 * b + 32],
                                         zt[0:1, 0:32])

                if b % 8 == 7:
                    evac_quarter(b // 8)
                if b == 15:
                    out_proj_half(0)

            # ---- hoisted batches' tails: only AV depends on the vt stream.
            # bc for quarter 3 is ready before the stream drains; the psav
            # columns of b24..30 evacuate behind AV(30), so after the final
            # vt transfer only AV(31) + 32 evac columns + projection remain.
            bc3 = make_bc(3)
            for b in HOIST:
                vt = stream.tile([128, 8, HC, DH], F8E3, name="vt", tag="vt",
                                 bufs=8)
                (nc.gpsimd if b % 2 == 0 else nc.sync).dma_start(
                    vt[0:P, :, :, :], v[b])
                m, r0 = b // 16, 8 * (b % 16)
                nc.gpsimd.dma_start(vt[P:128, :, :, :], vnew[m][r0:r0 + 8, :])
                for hp in range(2):
                    col = 32 * b + 16 * hp
                    for i in range(8):
                        nc.tensor.matmul(
                            psav[:, col:col + 16],
                            vt[:, i, 2 * hp:2 * hp + 2, :],
                            attnT_h[b][:, i, 16 * hp:16 * hp + 16],
                            start=(i == 0), stop=(i == 7))
                if b == B - 2:
                    evac_range(3, bc3, 0, 7)
            evac_range(3, bc3, 7, 8)
            out_proj_half(1)

    nc.finalize()
    return nc


def _prep_core(c, x_flat_T, cache_keys, cache_values, Wq, bq, Wk, bk, Wv, bv, Wo, bo):
    hs = slice(HC * c, HC * c + HC)
    qs = slice(QD * c, QD * c + QD)

    def perm_rows(W):
        # rows ordered (m, h, j): row 32h + j of tile m = W[64h + 32m + j]
        Ws = W[qs].reshape(HC, 2, 32, -1)              # [h, m, j, d]
        return Ws.transpose(1, 0, 2, 3).reshape(QD, -1)  # [(m,h,j), d]

    wq_p = perm_rows(Wq)
    wk_p = perm_rows(Wk)
    bq_p = np.ascontiguousarray(perm_rows(bq[:, None])[:, 0])
    bk_p = np.ascontiguousarray(perm_rows(bk[:, None])[:, 0])

    def as_tiles(WT):  # [D, 256] -> [128, 16, 256]
        return np.ascontiguousarray(
            WT.reshape(16, 128, QD).transpose(1, 0, 2)).astype(bfloat16)

    wqT = as_tiles(np.ascontiguousarray(wq_p.T))
    wkT = as_tiles(np.ascontiguousarray(wk_p.T))
    wvT = as_tiles(np.ascontiguousarray(Wv[qs].T))
    woT = np.ascontiguousarray(
        Wo[:, qs].T.reshape(2, 128, D).transpose(1, 0, 2)).astype(bfloat16)

    # kT[b, 32h+j, m, i, p] = K[b, h, 8p+i, 32m+j]; p=127 filled on device
    ck = cache_keys[:, hs]                        # [B, 4, 1016, 64]
    km = ck.reshape(B, HC, P, 8, 2, 32)           # [b, h, p, i, m, j]
    kT = np.zeros((B, HC, 32, 2, 8, 128), dtype=bfloat16)  # [b, h, j, m, i, p]
    kT[..., :P] = km.transpose(0, 1, 5, 4, 3, 2).astype(bfloat16)
    kT = kT.reshape(B, 128, 2, 8, 128)

    # v[b, p, i, h, d] = V[b, h, 8p+i, d]
    cv = cache_values[:, hs].reshape(B, HC, P, 8, DH)
    vv = np.ascontiguousarray(cv.transpose(0, 2, 3, 1, 4)).astype(float8_e3m4)

    return {
        "xT": x_flat_T.astype(bfloat16),
        "wqT": wqT, "wkT": wkT, "wvT": wvT, "woT": woT,
        "bq": bq_p.astype(np.float32), "bk": bk_p.astype(np.float32),
        "bv": np.ascontiguousarray(bv[qs]).astype(np.float32),
        "bo": bo.astype(bfloat16),
        "kT": kT,
        "v": vv,
    }


_NC_CACHE = {}


def kernel(x, cache_keys, cache_values, Wq, bq, Wk, bk, Wv, bv, Wo, bo):
    x = np.asarray(x, dtype=np.float32)
    cache_keys = np.asarray(cache_keys, dtype=np.float32)
    cache_values = np.asarray(cache_values, dtype=np.float32)
    Wq, Wk, Wv, Wo = (np.asarray(w, dtype=np.float32) for w in (Wq, Wk, Wv, Wo))
    bq, bk, bv, bo = (np.asarray(b_, dtype=np.float32) for b_ in (bq, bk, bv, bo))

    x_flat_T = np.ascontiguousarray(
        x.reshape(TOK, D).T.reshape(16, 128, TOK).transpose(1, 0, 2))  # [128,16,256]

    in_maps = [
        _prep_core(c, x_flat_T, cache_keys, cache_values,
                   Wq, bq, Wk, bk, Wv, bv, Wo, bo)
        for c in range(N_CORES)
    ]

    cfg = dict(CFG)
    cfg["bo_zero"] = not np.any(bo)
    key = tuple(sorted(cfg.items()))
    if key not in _NC_CACHE:
        _NC_CACHE[key] = build_nc(cfg)
    nc = _NC_CACHE[key]

    res = bass_utils.run_bass_kernel_spmd(nc, in_maps, core_ids=list(range(N_CORES)))
    out = np.zeros((TOK, D), dtype=np.float32)
    for r in res.results:
        out += r["out"].astype(np.float32)
    return out.reshape(B, T, D)
